# revision 2
# baseline (speedup 1.0000x reference)
"""MBConv (4D spatial, 16^4) on 8 TRN2 NeuronCores.

Sharding: spatial-parallel over the first spatial dim X (16 planes ->
2 owned planes per core + 1 halo plane each side, shipped from host).

Math (all on device except weight-only constant folding on host):
  GN0+conv1+GN1 folded: A' = (W1 * g0_w) . x computed once; the two
  global groupnorms reduce to 6 scalars in ONE AllReduce:
    [Sum(A'), Sum(A'^2), Sum(u*SA), Sum(v*SA), Sum(x), Sum(x^2)]
  with u = W1.g0_b, v = W1.g0_w (host constants); then
  h1 = gelu(alpha1 * A' + beta1) per hidden channel.
  conv2 = 81 accumulating PE matmuls per PSUM bank over a zero-padded
  [128ch, 4planes, 18,18,18] SBUF tile (float32r fast-fp32 mode).
  GN2 -> AllReduce(2 scalars); gelu fused with SE partial-mean accum.
  SE mean -> AllReduce(128); SE MLP on-device; scale folded into w3.
  conv3; GN3 -> AllReduce(2 scalars); affine; DMA out.
"""

import sys
sys.path.insert(0, '/opt/trn_rl_repo')

import numpy as np
import ml_dtypes

import concourse.bass as bass
import concourse.bacc as bacc
import concourse.tile as tile
import concourse.mybir as mybir
from concourse.bass_utils import run_bass_kernel_spmd

F32 = mybir.dt.float32
F32R = mybir.dt.float32r
BF16 = mybir.dt.bfloat16
AF = mybir.ActivationFunctionType

N_CORES = 8
S = 16
CIN = 32
HID = 128
EPS = 1e-5
PLANE = S * S * S            # 4096 positions per x-plane
PPAD = 18 * 18 * 18          # padded plane (z/y/w pad 1)
NPL = 4                      # stored planes per core (2 owned + 2 halo)
POS = 2 * PLANE              # owned positions per core
P_SP = S ** 4                # 65536 global spatial positions
NX = CIN * P_SP
N1 = HID * P_SP
N3 = CIN * P_SP

_cache = {}


def _col(t, i):
    return t[:, i:i + 1]


def build_program(trace_scopes=False):
    nc = bacc.Bacc("TRN2", target_bir_lowering=False, debug=False,
                   enable_asserts=False, num_devices=N_CORES)

    xs_d = nc.dram_tensor("xs", [128, PLANE], F32R, kind="ExternalInput").ap()
    w1_d = nc.dram_tensor("w1rep", [128, 128], F32R, kind="ExternalInput").ap()
    w2_d = nc.dram_tensor("w2t", [128, 81 * 128], BF16, kind="ExternalInput").ap()
    pp_d = nc.dram_tensor("params", [128, 192], F32, kind="ExternalInput").ap()
    out_d = nc.dram_tensor("out", [CIN, POS], F32, kind="ExternalOutput").ap()

    with tile.TileContext(nc) as tc:
        with tc.tile_pool(name="big", bufs=1) as big, \
             tc.tile_pool(name="small", bufs=1) as small, \
             tc.tile_pool(name="scr", bufs=24) as scr, \
             tc.tile_pool(name="ps", bufs=8, space="PSUM") as ps, \
             tc.tile_pool(name="dram", bufs=1, space="DRAM") as dram:

            def stile(shape, name, pool=None):
                return (pool or small).tile(shape, F32, name=name)

            def sc(name):
                return scr.tile([128, 1], F32, tag="scr", name=name)

            # ---- persistent SBUF tensors ----
            x_sb = big.tile([128, PLANE], F32R, name="x_sb")
            w1_sb = big.tile([128, 128], F32R, name="w1_sb")
            w2_sb = big.tile([128, 81 * 128], BF16, name="w2_sb")
            pp = big.tile([128, 192], F32, name="pp")
            h1 = big.tile([128, NPL * PPAD], BF16, name="h1", tag="bigslot")
            h2 = big.tile([128, 2 * PLANE], F32R, name="h2")

            # small weights first (conv1's first matmul needs w1 + x chunk 0),
            # then x per-plane chunks so conv1 starts while later planes load
            nc.sync.dma_start(out=w1_sb, in_=w1_d)
            nc.sync.dma_start(out=pp, in_=pp_d)
            for sj in range(NPL):
                nc.sync.dma_start(out=x_sb[32 * sj:32 * sj + 32, :],
                                  in_=xs_d[32 * sj:32 * sj + 32, :])
            nc.sync.dma_start(out=w2_sb, in_=w2_d)

            # AR bounce buffers: pre-zero the pad lanes once, off the
            # critical path; stats are later DMA'd straight from PSUM
            d1i = dram.tile([8], F32, name="d1i")
            d2i = dram.tile([8], F32, name="d2i")
            d4i = dram.tile([8], F32, name="d4i")
            zrow = small.tile([1, 8], F32, name="zrow")
            nc.vector.memset(zrow, 0.0)
            nc.sync.dma_start(out=d1i, in_=zrow)
            nc.sync.dma_start(out=d2i, in_=zrow)
            nc.sync.dma_start(out=d4i, in_=zrow)

            h1f5 = h1.rearrange("p (j y z w) -> p j y z w", j=NPL, y=18, z=18, w=18)
            h1pl = h1.rearrange("p (j r) -> p j r", j=NPL, r=PPAD)
            # zero h1 (padding must be 0)
            for j in range(NPL):
                eng = nc.vector if j % 2 == 0 else nc.gpsimd
                eng.memset(h1pl[:, j, :], 0.0)

            def interior(j):
                return h1f5[:, j, 1:17, 1:17, 1:17]

            def interior_chunk(j, n):  # output y-pair chunk [128,(2,16,16)]
                return h1f5[:, j, 1 + 2 * n:3 + 2 * n, 1:17, 1:17]

            eps_t = stile([128, 1], "eps_t")
            nc.vector.memset(eps_t, EPS)
            ones = stile([128, 1], "ones")
            nc.vector.memset(ones, 1.0)

            # ---- conv1: A' = (W1*g0w) . x  on all 4 planes ----
            # Shard partition packing puts OWNED planes on partitions 0:64
            # (stored order [owned0, owned1, haloL, haloR]); LOC maps stored
            # plane index -> local x position in the padded h1 buffer.
            # A'-stats (owned planes only) taken from the contiguous PSUM
            # tiles before eviction (bn_stats reduces innermost dim only).
            LOC = (1, 2, 0, 3)
            # stage A' contiguously; h1 keeps few writers (memset+gelu+mask)
            # so conv2's dependency tracking stays cheap
            aprime = big.tile([128, NPL * PLANE], BF16, name="aprime")
            ap5 = aprime.rearrange("p (s y z w) -> p s y z w",
                                   s=NPL, y=16, z=16, w=16)
            sta = stile([128, 16, 6], "sta")
            for sj in range(NPL):
                for n in range(8):
                    pt = ps.tile([128, 512], F32, tag="ps", name=f"c1_{sj}_{n}")
                    nc.tensor.matmul(
                        pt,
                        w1_sb[32 * sj:32 * sj + 32, :],
                        x_sb[32 * sj:32 * sj + 32, bass.ts(n, 512)],
                        start=True, stop=True, tile_position=(32 * sj, 0))
                    nc.scalar.copy(
                        out=aprime[:, bass.ts(sj * 8 + n, 512)], in_=pt)
                    if sj < 2:
                        nc.vector.bn_stats(out=sta[:, sj * 8 + n, :], in_=pt)

            # ---- stats for folded GN0+GN1 (owned data only) ----
            stx = stile([128, 8, 6], "stx")
            x_f32 = x_sb.bitcast(F32)
            for c in range(8):
                nc.vector.bn_stats(out=stx[0:64, c, :],
                                   in_=x_f32[0:64, bass.ts(c, 512)])
            mvx = stile([128, 2], "mvx")
            nc.vector.bn_aggr(out=mvx[0:64, :], in_=stx[0:64])

            mva = stile([128, 2], "mva")
            nc.vector.bn_aggr(out=mva, in_=sta)

            pk = stile([128, 6], "pk")
            nc.vector.memset(pk, 0.0)
            # col0: SA_o = mean*POS ; col1: SAA_o = (var+mean^2)*POS
            nc.vector.tensor_scalar_mul(out=_col(pk, 0), in0=_col(mva, 0), scalar1=float(POS))
            t_a = sc("t_a")
            nc.vector.tensor_mul(t_a, _col(mva, 0), _col(mva, 0))
            nc.vector.tensor_add(t_a, t_a, _col(mva, 1))
            nc.vector.tensor_scalar_mul(out=_col(pk, 1), in0=t_a, scalar1=float(POS))
            nc.vector.tensor_mul(_col(pk, 2), _col(pp, 0), _col(pk, 0))   # u*SA
            nc.vector.tensor_mul(_col(pk, 3), _col(pp, 1), _col(pk, 0))   # v*SA
            # x stats on owned planes (partitions 0:64, 4096 positions each)
            nc.vector.tensor_scalar_mul(out=pk[0:64, 4:5], in0=mvx[0:64, 0:1], scalar1=float(PLANE))
            t_b = sc("t_b")
            nc.vector.tensor_mul(t_b[0:64], mvx[0:64, 0:1], mvx[0:64, 0:1])
            nc.vector.tensor_add(t_b[0:64], t_b[0:64], mvx[0:64, 1:2])
            nc.vector.tensor_scalar_mul(out=pk[0:64, 5:6], in0=t_b[0:64], scalar1=float(PLANE))

            ps_s1 = ps.tile([1, 6], F32, tag="ps", name="ps_s1")
            nc.tensor.matmul(ps_s1, ones, pk, start=True, stop=True)
            d1o = dram.tile([8], F32, name="d1o")
            row1 = stile([1, 6], "row1")
            nc.vector.tensor_copy(out=row1, in_=ps_s1)
            nc.sync.dma_start(out=d1i[0:6], in_=row1)
            nc.gpsimd.collective_compute(
                "AllReduce", mybir.AluOpType.add,
                replica_groups=[list(range(N_CORES))],
                ins=[d1i.opt()], outs=[d1o.opt()])
            g1 = stile([128, 8], "g1")
            nc.sync.dma_start(out=g1, in_=bass.AP(
                tensor=d1o.tensor, offset=d1o.offset, ap=[[0, 128]] + list(d1o.ap)))

            # ---- scalar chain (replicated on 128 partitions) ----
            def gn_mu_r(g, i_sum, i_ss, nval, tag):
                mu = stile([128, 1], f"mu_{tag}")
                nc.vector.tensor_scalar_mul(out=mu, in0=_col(g, i_sum), scalar1=1.0 / nval)
                ex2 = sc(f"ex2_{tag}")
                nc.vector.tensor_scalar_mul(out=ex2, in0=_col(g, i_ss), scalar1=1.0 / nval)
                var = sc(f"var_{tag}")
                nc.vector.tensor_mul(var, mu, mu)
                nc.vector.tensor_sub(var, ex2, var)
                std = sc(f"std_{tag}")
                nc.scalar.activation(out=std, in_=var, func=AF.Sqrt, bias=eps_t)
                r = stile([128, 1], f"r_{tag}")
                nc.vector.reciprocal(r, std)
                return mu, r

            # g1 cols: 0 SumSA, 1 SAA, 2 SumU.SA, 3 SumV.SA, 4 Sx, 5 Sxx
            mu0, r0 = gn_mu_r(g1, 4, 5, NX, "0")
            q = stile([128, 1], "q")
            nc.vector.tensor_mul(q, mu0, r0)
            scsa = sc("scsa")                       # Sum(c*SA) = col2 - q*col3
            nc.vector.tensor_mul(scsa, q, _col(g1, 3))
            nc.vector.tensor_sub(scsa, _col(g1, 2), scsa)
            s_c = sc("s_c")                         # Sum(c) = Su - q*Sv
            nc.vector.tensor_mul(s_c, q, _col(pp, 11))
            nc.vector.tensor_sub(s_c, _col(pp, 10), s_c)
            scc = sc("scc")                         # Sum(c^2)
            t_c = sc("t_c")
            nc.vector.tensor_mul(t_c, q, _col(pp, 13))
            nc.vector.tensor_scalar_mul(out=t_c, in0=t_c, scalar1=2.0)
            nc.vector.tensor_sub(scc, _col(pp, 12), t_c)
            nc.vector.tensor_mul(t_c, q, q)
            nc.vector.tensor_mul(t_c, t_c, _col(pp, 14))
            nc.vector.tensor_add(scc, scc, t_c)
            # mu1
            mu1 = stile([128, 1], "mu1")
            nc.vector.tensor_mul(mu1, r0, _col(g1, 0))
            t_d = sc("t_d")
            nc.vector.tensor_scalar_mul(out=t_d, in0=s_c, scalar1=float(P_SP))
            nc.vector.tensor_add(mu1, mu1, t_d)
            nc.vector.tensor_scalar_mul(out=mu1, in0=mu1, scalar1=1.0 / N1)
            # var1 = (r0^2*SAA + 2 r0 scsa + P*scc)/N1 - mu1^2
            v1 = sc("v1")
            nc.vector.tensor_mul(v1, r0, r0)
            nc.vector.tensor_mul(v1, v1, _col(g1, 1))
            t_e = sc("t_e")
            nc.vector.tensor_mul(t_e, r0, scsa)
            nc.vector.tensor_scalar_mul(out=t_e, in0=t_e, scalar1=2.0)
            nc.vector.tensor_add(v1, v1, t_e)
            nc.vector.tensor_scalar_mul(out=t_e, in0=scc, scalar1=float(P_SP))
            nc.vector.tensor_add(v1, v1, t_e)
            nc.vector.tensor_scalar_mul(out=v1, in0=v1, scalar1=1.0 / N1)
            nc.vector.tensor_mul(t_e, mu1, mu1)
            nc.vector.tensor_sub(v1, v1, t_e)
            std1 = sc("std1")
            nc.scalar.activation(out=std1, in_=v1, func=AF.Sqrt, bias=eps_t)
            r1 = stile([128, 1], "r1")
            nc.vector.reciprocal(r1, std1)
            al1 = stile([128, 1], "al1")
            nc.vector.tensor_mul(al1, r0, r1)
            nc.vector.tensor_mul(al1, al1, _col(pp, 2))
            be1 = stile([128, 1], "be1")
            nc.vector.tensor_mul(be1, q, _col(pp, 1))        # q*v
            nc.vector.tensor_sub(be1, _col(pp, 0), be1)      # c = u - q*v
            nc.vector.tensor_sub(be1, be1, mu1)              # c - mu1
            nc.vector.tensor_mul(be1, be1, r1)
            nc.vector.tensor_mul(be1, be1, _col(pp, 2))
            nc.vector.tensor_add(be1, be1, _col(pp, 3))

            # ---- h1 = gelu(alpha1*A' + beta1); mask edge halos ----
            # order: haloL first+mask, then owned planes, then haloR — conv2's
            # first output plane needs local planes 0..2; plane 3 gelu
            # overlaps conv2's start
            for sj in (2, 0, 1, 3):
                lj = LOC[sj]
                nc.scalar.activation(out=interior(lj), in_=ap5[:, sj],
                                     func=AF.Gelu, bias=be1, scale=al1)
                if lj == 0:
                    nc.vector.tensor_scalar_mul(out=interior(0), in0=interior(0),
                                                scalar1=_col(pp, 8))
                elif lj == NPL - 1:
                    nc.gpsimd.tensor_scalar_mul(out=interior(NPL - 1),
                                                in0=interior(NPL - 1),
                                                scalar1=_col(pp, 9))

            # ---- conv2: 3^4, 81 taps, accumulate in PSUM ----
            h1r5 = h1f5
            w2r = w2_sb
            sth = stile([128, 16, 6], "sth")
            for j in range(2):
                # taps-outer: load each tap's weights ONCE per output plane,
                # then stream all 8 banks with the same stationary weights —
                # avoids the per-weight-change PE drain (~45ns) 8x per tap
                pts = [ps.tile([128, 512], F32, tag="ps", name=f"c2_{j}_{b}")
                       for b in range(8)]
                t = 0
                for dx in range(3):
                    for dy in range(3):
                        for dz in range(3):
                            for dw in range(3):
                                for b in range(8):
                                    mov = h1r5[:, j + dx,
                                               2 * b + dy:2 * b + dy + 2,
                                               dz:dz + 16, dw:dw + 16]
                                    nc.tensor.matmul(pts[b], w2r[:, bass.ts(t, 128)],
                                                     mov,
                                                     start=(t == 0), stop=(t == 80))
                                t += 1
                for b in range(8):
                    blk = bass.ts(j * 8 + b, 512)
                    nc.scalar.copy(out=h2[:, blk], in_=pts[b])
                    nc.vector.bn_stats(out=sth[:, j * 8 + b, :],
                                       in_=h2.bitcast(F32)[:, blk])

            mvh = stile([128, 2], "mvh")
            nc.vector.bn_aggr(out=mvh, in_=sth)
            pk2 = stile([128, 2], "pk2")
            nc.vector.tensor_scalar_mul(out=_col(pk2, 0), in0=_col(mvh, 0), scalar1=float(POS))
            t_f = sc("t_f")
            nc.vector.tensor_mul(t_f, _col(mvh, 0), _col(mvh, 0))
            nc.vector.tensor_add(t_f, t_f, _col(mvh, 1))
            nc.vector.tensor_scalar_mul(out=_col(pk2, 1), in0=t_f, scalar1=float(POS))
            ps_s2 = ps.tile([1, 2], F32, tag="ps", name="ps_s2")
            nc.tensor.matmul(ps_s2, ones, pk2, start=True, stop=True)
            d2o = dram.tile([8], F32, name="d2o")
            row2 = stile([1, 2], "row2")
            nc.vector.tensor_copy(out=row2, in_=ps_s2)
            nc.sync.dma_start(out=d2i[0:2], in_=row2)
            nc.gpsimd.collective_compute(
                "AllReduce", mybir.AluOpType.add,
                replica_groups=[list(range(N_CORES))],
                ins=[d2i.opt()], outs=[d2o.opt()])
            g2 = stile([128, 8], "g2")
            nc.sync.dma_start(out=g2, in_=bass.AP(
                tensor=d2o.tensor, offset=d2o.offset, ap=[[0, 128]] + list(d2o.ap)))

            mu2, r2 = gn_mu_r(g2, 0, 1, N1, "2")
            al2 = stile([128, 1], "al2")
            nc.vector.tensor_mul(al2, r2, _col(pp, 4))
            be2 = stile([128, 1], "be2")
            nc.vector.tensor_mul(be2, mu2, al2)
            nc.vector.tensor_sub(be2, _col(pp, 5), be2)

            # ---- gelu(GN2) in place + SE partial sums via accum_out ----
            mcols = stile([128, 16], "mcols")
            h2f = h2.bitcast(F32)
            for n in range(16):
                nc.scalar.activation(out=h2[:, bass.ts(n, 512)],
                                     in_=h2f[:, bass.ts(n, 512)],
                                     func=AF.Gelu, bias=be2, scale=al2,
                                     accum_out=mcols[:, n:n + 1])
            m_col = stile([128, 1], "m_col")
            nc.vector.reduce_sum(out=m_col, in_=mcols, axis=mybir.AxisListType.X)
            d3i = dram.tile([128], F32, name="d3i")
            d3o = dram.tile([128], F32, name="d3o")
            nc.sync.dma_start(out=d3i, in_=m_col)
            nc.gpsimd.collective_compute(
                "AllReduce", mybir.AluOpType.add,
                replica_groups=[list(range(N_CORES))],
                ins=[d3i.opt()], outs=[d3o.opt()])
            m_sb = stile([128, 1], "m_sb")
            nc.sync.dma_start(out=m_sb, in_=d3o)

            # ---- SE MLP (tiny, replicated on every core) ----
            m_mean = stile([128, 1], "m_mean")
            nc.vector.tensor_scalar_mul(out=m_mean, in0=m_sb, scalar1=1.0 / P_SP)
            ps_se1 = ps.tile([8, 1], F32, tag="ps", name="ps_se1")
            nc.tensor.matmul(ps_se1, pp[:, 16:24], m_mean, start=True, stop=True)
            y1g = stile([8, 1], "y1g")
            nc.scalar.activation(out=y1g, in_=ps_se1, func=AF.Gelu)
            ps_se2 = ps.tile([128, 1], F32, tag="ps", name="ps_se2")
            nc.tensor.matmul(ps_se2, pp[0:8, 56:184], y1g, start=True, stop=True)
            s_sb = stile([128, 1], "s_sb")
            nc.scalar.activation(out=s_sb, in_=ps_se2, func=AF.Sigmoid)
            w3s = small.tile([128, 32], F32R, name="w3s")
            nc.vector.tensor_scalar_mul(out=w3s, in0=pp[:, 24:56], scalar1=s_sb)

            # ---- conv3 (+ stats), y3 shares the h1 slot ----
            y3 = big.tile([CIN, POS], F32, name="y3", tag="bigslot")
            st3 = stile([32, 16, 6], "st3")
            for n in range(16):
                pt3 = ps.tile([32, 512], F32, tag="ps", name=f"c3_{n}")
                nc.tensor.matmul(pt3, w3s, h2[:, bass.ts(n, 512)],
                                 start=True, stop=True)
                nc.scalar.copy(out=y3[:, bass.ts(n, 512)], in_=pt3)
                nc.vector.bn_stats(out=st3[:, n, :], in_=pt3)
            mv3 = stile([32, 2], "mv3")
            nc.vector.bn_aggr(out=mv3, in_=st3)
            pk3 = stile([128, 2], "pk3")
            nc.vector.memset(pk3, 0.0)
            nc.vector.tensor_scalar_mul(out=pk3[0:32, 0:1], in0=mv3[:, 0:1], scalar1=float(POS))
            t_g = sc("t_g")
            nc.vector.tensor_mul(t_g[0:32], mv3[:, 0:1], mv3[:, 0:1])
            nc.vector.tensor_add(t_g[0:32], t_g[0:32], mv3[:, 1:2])
            nc.vector.tensor_scalar_mul(out=pk3[0:32, 1:2], in0=t_g[0:32], scalar1=float(POS))
            ps_s3 = ps.tile([1, 2], F32, tag="ps", name="ps_s3")
            nc.tensor.matmul(ps_s3, ones, pk3, start=True, stop=True)
            d4o = dram.tile([8], F32, name="d4o")
            row3 = stile([1, 2], "row3")
            nc.vector.tensor_copy(out=row3, in_=ps_s3)
            nc.sync.dma_start(out=d4i[0:2], in_=row3)
            nc.gpsimd.collective_compute(
                "AllReduce", mybir.AluOpType.add,
                replica_groups=[list(range(N_CORES))],
                ins=[d4i.opt()], outs=[d4o.opt()])
            g4 = stile([128, 8], "g4")
            nc.sync.dma_start(out=g4, in_=bass.AP(
                tensor=d4o.tensor, offset=d4o.offset, ap=[[0, 128]] + list(d4o.ap)))

            mu3, r3 = gn_mu_r(g4, 0, 1, N3, "3")
            al3 = stile([128, 1], "al3")
            nc.vector.tensor_mul(al3, r3, _col(pp, 6))
            be3 = stile([128, 1], "be3")
            nc.vector.tensor_mul(be3, mu3, al3)
            nc.vector.tensor_sub(be3, _col(pp, 7), be3)

            # final affine in 4 chunks across two engines; each chunk's store
            # DMA starts as soon as that chunk is done
            qn = POS // 4
            for q in range(4):
                eng = nc.vector if q % 2 == 0 else nc.gpsimd
                blk = slice(q * qn, (q + 1) * qn)
                eng.tensor_scalar(out=y3[:, blk], in0=y3[:, blk],
                                  scalar1=al3[0:32], scalar2=be3[0:32],
                                  op0=mybir.AluOpType.mult,
                                  op1=mybir.AluOpType.add)
                nc.sync.dma_start(out=out_d[:, blk], in_=y3[:, blk])

    nc.compile()
    return nc


def _host_prep(inputs):
    x = np.asarray(inputs['x'], np.float32).reshape(CIN, S, S, S, S)
    g0w = np.asarray(inputs['g0_w'], np.float32)
    g0b = np.asarray(inputs['g0_b'], np.float32)
    W1 = np.asarray(inputs['w1'], np.float32).reshape(HID, CIN)
    gn1w = np.asarray(inputs['gn1_w'], np.float32)
    gn1b = np.asarray(inputs['gn1_b'], np.float32)
    w2 = np.asarray(inputs['w2'], np.float32).reshape(HID, HID, 3, 3, 3, 3)
    gn2w = np.asarray(inputs['gn2_w'], np.float32)
    gn2b = np.asarray(inputs['gn2_b'], np.float32)
    se1 = np.asarray(inputs['se_w1'], np.float32)   # [8,128]
    se2 = np.asarray(inputs['se_w2'], np.float32)   # [128,8]
    W3 = np.asarray(inputs['w3'], np.float32).reshape(CIN, HID)
    gn3w = np.asarray(inputs['gn3_w'], np.float32)
    gn3b = np.asarray(inputs['gn3_b'], np.float32)

    w1fold = W1 * g0w[None, :]
    w1rep = np.zeros((128, 128), np.float32)
    for j in range(4):
        w1rep[32 * j:32 * j + 32, :] = w1fold.T
    u = W1 @ g0b
    v = W1 @ g0w
    w2t = np.ascontiguousarray(
        w2.transpose(1, 2, 3, 4, 5, 0).reshape(HID, 81 * HID)).astype(
            ml_dtypes.bfloat16)

    params = np.zeros((128, 192), np.float32)
    params[:, 0] = u
    params[:, 1] = v
    params[:, 2] = gn1w
    params[:, 3] = gn1b
    params[:, 4] = gn2w
    params[:, 5] = gn2b
    params[0:32, 6] = gn3w
    params[0:32, 7] = gn3b
    params[:, 10] = u.sum()
    params[:, 11] = v.sum()
    params[:, 12] = (u * u).sum()
    params[:, 13] = (u * v).sum()
    params[:, 14] = (v * v).sum()
    params[:, 16:24] = se1.T
    params[:, 24:56] = W3.T
    params[0:8, 56:184] = se2.T

    xp = np.zeros((CIN, S + 2, S, S, S), np.float32)
    xp[:, 1:S + 1] = x

    in_maps = []
    for k in range(N_CORES):
        p = params.copy()
        p[:, 8] = 0.0 if k == 0 else 1.0
        p[:, 9] = 0.0 if k == N_CORES - 1 else 1.0
        # stored plane order: [owned0, owned1, haloL, haloR]
        idx = [2 * k + 1, 2 * k + 2, 2 * k, 2 * k + 3]
        shard = np.ascontiguousarray(
            xp[:, idx].transpose(1, 0, 2, 3, 4).reshape(128, PLANE))
        in_maps.append({"xs": shard, "w1rep": w1rep, "w2t": w2t, "params": p})
    return in_maps


def kernel(**inputs):
    if "nc" not in _cache:
        _cache["nc"] = build_program()
    nc = _cache["nc"]
    in_maps = _host_prep(inputs)
    res = run_bass_kernel_spmd(nc, in_maps, core_ids=list(range(N_CORES)))
    out = np.empty((1, CIN, S, S, S, S), np.float32)
    for k in range(N_CORES):
        out[0, :, 2 * k:2 * k + 2] = res.results[k]["out"].reshape(CIN, 2, S, S, S)
    return out


def run_traced(inputs):
    """Like kernel() but with NTFF tracing; returns (out, BassKernelResults)."""
    if "nc" not in _cache:
        _cache["nc"] = build_program()
    nc = _cache["nc"]
    in_maps = _host_prep(inputs)
    res = run_bass_kernel_spmd(nc, in_maps, core_ids=list(range(N_CORES)),
                               trace=True)
    out = np.empty((1, CIN, S, S, S, S), np.float32)
    for k in range(N_CORES):
        out[0, :, 2 * k:2 * k + 2] = res.results[k]["out"].reshape(CIN, 2, S, S, S)
    return out, res



# revision 11
# speedup vs baseline: 1.0056x; 1.0056x over previous
"""MBConv (4D spatial, 16^4) on 8 TRN2 NeuronCores.

Sharding: spatial-parallel over the first spatial dim X (16 planes ->
2 owned planes per core + 1 halo plane each side, shipped from host).

Math (all on device except weight-only constant folding on host):
  GN0+conv1+GN1 folded: A' = (W1 * g0_w) . x computed once; the two
  global groupnorms reduce to 6 scalars in ONE AllReduce:
    [Sum(A'), Sum(A'^2), Sum(u*SA), Sum(v*SA), Sum(x), Sum(x^2)]
  with u = W1.g0_b, v = W1.g0_w (host constants); then
  h1 = gelu(alpha1 * A' + beta1) per hidden channel.
  conv2 = 81 accumulating PE matmuls per PSUM bank over a zero-padded
  [128ch, 4planes, 18,18,18] SBUF tile (bf16).
  GN2 -> AllReduce(2 scalars); gelu fused with SE mean via accum_out.
  SE mean -> AllReduce(128); SE MLP on-device; scale folded into w3.
  conv3 (bf16); GN3 -> AllReduce(2 scalars); affine; DMA out.

v2 perf structure:
  - warmup AllReduce at t=0 absorbs CC-engine startup + core skew
  - x/conv1/conv3 in bf16 (fp32r matmuls run ~2x slow on HW)
  - conv1 owned planes first; AR1 launches while halo conv1 runs
  - rsqrt via int bit-trick + 2 Newton steps on DVE: the scalar engine
    keeps the Gelu table loaded -> no ACT_TABLE_LOAD on critical path
  - halo masks folded into gelu scale/bias (gelu(0*x+0) == 0)
  - gelu h1 in half-planes ordered to unblock conv2 bank 0 early
  - gelu h2 one-shot [128,8192] with accum_out = SE partial mean
"""

import sys
sys.path.insert(0, '/opt/trn_rl_repo')

import numpy as np
import ml_dtypes

import concourse.bass as bass
import concourse.bacc as bacc
import concourse.tile as tile
import concourse.mybir as mybir
from concourse.bass_utils import run_bass_kernel_spmd

F32 = mybir.dt.float32
I32 = mybir.dt.int32
BF16 = mybir.dt.bfloat16
AF = mybir.ActivationFunctionType
ALU = mybir.AluOpType

N_CORES = 8
S = 16
CIN = 32
HID = 128
EPS = 1e-5
PLANE = S * S * S            # 4096 positions per x-plane
PPAD = 18 * 18 * 18          # padded plane (z/y/w pad 1)
NPL = 4                      # stored planes per core (2 owned + 2 halo)
POS = 2 * PLANE              # owned positions per core
P_SP = S ** 4                # 65536 global spatial positions
NX = CIN * P_SP
N1 = HID * P_SP
N3 = CIN * P_SP

_cache = {}


def _col(t, i):
    return t[:, i:i + 1]


def build_program(trace_scopes=False):
    nc = bacc.Bacc("TRN2", target_bir_lowering=False, debug=False,
                   enable_asserts=False, num_devices=N_CORES)

    xs_d = nc.dram_tensor("xs", [128, PLANE], BF16, kind="ExternalInput").ap()
    w1_d = nc.dram_tensor("w1rep", [128, 128], BF16, kind="ExternalInput").ap()
    w2_d = nc.dram_tensor("w2t", [128, 81 * 128], BF16, kind="ExternalInput").ap()
    pp_d = nc.dram_tensor("params", [128, 192], F32, kind="ExternalInput").ap()
    id_d = nc.dram_tensor("ident", [128, 128], F32, kind="ExternalInput").ap()
    out_d = nc.dram_tensor("out", [CIN, POS], F32, kind="ExternalOutput").ap()

    with tile.TileContext(nc) as tc:
        with tc.tile_pool(name="big", bufs=1) as big, \
             tc.tile_pool(name="small", bufs=1) as small, \
             tc.tile_pool(name="scr", bufs=48) as scr, \
             tc.tile_pool(name="ps", bufs=8, space="PSUM") as ps, \
             tc.tile_pool(name="dram", bufs=1, space="DRAM") as dram:

            def stile(shape, name, pool=None, dtype=F32):
                return (pool or small).tile(shape, dtype, name=name)

            def sc(name, dtype=F32):
                return scr.tile([128, 1], dtype, tag="scr", name=name)

            # ---- persistent SBUF tensors ----
            x_sb = big.tile([128, PLANE], BF16, name="x_sb")
            w1_sb = big.tile([128, 128], BF16, name="w1_sb")
            w2_sb = big.tile([128, 81 * 128], BF16, name="w2_sb")
            pp = big.tile([128, 192], F32, name="pp")
            h1 = big.tile([128, NPL * PPAD], BF16, name="h1", tag="h1slot")
            h2 = big.tile([128, 2 * PLANE], F32, name="h2")
            h2b = big.tile([128, 2 * PLANE], BF16, name="h2b")
            # aprime (conv1 staging) later reused for the final f32 output
            aprime = big.tile([128, NPL * PLANE], BF16, name="aprime",
                              tag="apslot")

            # The warmup collective fires FIRST: its input DMA precedes all
            # big transfers so the trigger lands at ~1us. It absorbs the
            # ~11us CC-engine first-collective startup and the inter-core
            # program-start skew (it acts as a barrier), so the real ARs
            # see only mesh-hop latency.
            dwi = dram.tile([8], F32, name="dwi")
            dwo = dram.tile([8], F32, name="dwo")
            d1i = dram.tile([8], F32, name="d1i")
            d2i = dram.tile([8], F32, name="d2i")
            d4i = dram.tile([8], F32, name="d4i")
            zrow = small.tile([1, 8], F32, name="zrow")
            nc.vector.memset(zrow, 0.0)
            nc.sync.dma_start(out=dwi, in_=zrow)
            nc.gpsimd.collective_compute(
                "AllReduce", mybir.AluOpType.add,
                replica_groups=[list(range(N_CORES))],
                ins=[dwi.opt()], outs=[dwo.opt()])

            # weights next (conv1's first matmul needs w1 + x chunk 0),
            # then x owned planes, then the rest
            nc.sync.dma_start(out=w1_sb, in_=w1_d)
            for sj in range(2):
                nc.sync.dma_start(out=x_sb[32 * sj:32 * sj + 32, :],
                                  in_=xs_d[32 * sj:32 * sj + 32, :])
            nc.sync.dma_start(out=pp, in_=pp_d)
            for sj in range(2, NPL):
                nc.sync.dma_start(out=x_sb[32 * sj:32 * sj + 32, :],
                                  in_=xs_d[32 * sj:32 * sj + 32, :])
            nc.sync.dma_start(out=w2_sb, in_=w2_d)
            id_sb = big.tile([128, 128], F32, name="id_sb")
            nc.sync.dma_start(out=id_sb, in_=id_d)
            nc.sync.dma_start(out=d1i, in_=zrow)
            nc.sync.dma_start(out=d2i, in_=zrow)
            nc.sync.dma_start(out=d4i, in_=zrow)

            # preload the activation tables used (Copy/Sigmoid/Gelu) while
            # the scalar engine is idle (each first use otherwise costs a
            # 1.3us ACT_TABLE_LOAD, some on the critical path). Gelu last.
            dummy = stile([1, 1], "dummy")
            nc.vector.memset(dummy, 0.0)
            nc.scalar.activation(out=dummy, in_=dummy, func=AF.Sigmoid)
            nc.scalar.copy(out=dummy, in_=dummy)
            nc.scalar.activation(out=dummy, in_=dummy, func=AF.Gelu)

            h1f5 = h1.rearrange("p (j y z w) -> p j y z w", j=NPL, y=18, z=18, w=18)
            h1pl = h1.rearrange("p (j r) -> p j r", j=NPL, r=PPAD)
            # zero h1 (padding must be 0); gelu-consumption order is local
            # planes 0(hL),1,2,3(hR): gpsimd zeroes 0,1; vector zeroes 2,3
            # after its stats work
            nc.gpsimd.memset(h1pl[:, 0, :], 0.0)
            nc.gpsimd.memset(h1pl[:, 1, :], 0.0)

            def interior(j):
                return h1f5[:, j, 1:17, 1:17, 1:17]

            ones = stile([128, 1], "ones")
            nc.vector.memset(ones, 1.0)

            # ---- DVE rsqrt: y = 1/sqrt(v) via bit trick + 2 Newton steps.
            # Keeps the scalar engine's Gelu table resident (no Sqrt table).
            def rsqrt_dve(out, v, tag):
                tb = sc(f"rs_i_{tag}", I32)
                vb = v.bitcast(I32)
                nc.vector.tensor_scalar(out=tb, in0=vb, scalar1=1,
                                        scalar2=None,
                                        op0=ALU.logical_shift_right)
                # magic - (v>>1), via subtract then negate (the fused
                # xor+add int form crashes the walrus backend)
                nc.vector.tensor_scalar(out=tb, in0=tb, scalar1=0x5f3759df,
                                        scalar2=None, op0=ALU.subtract)
                nc.vector.tensor_scalar(out=tb, in0=tb, scalar1=-1,
                                        scalar2=None, op0=ALU.mult)
                y = tb.bitcast(F32)
                h = sc(f"rs_h_{tag}")
                nc.vector.tensor_scalar_mul(out=h, in0=v, scalar1=0.5)
                t2 = sc(f"rs_t_{tag}")
                for it in range(2):
                    dst = out if it == 1 else y
                    nc.vector.tensor_mul(t2, y, y)
                    nc.vector.tensor_mul(t2, t2, h)
                    nc.vector.tensor_scalar(out=t2, in0=t2, scalar1=-1.0,
                                            scalar2=1.5, op0=ALU.mult,
                                            op1=ALU.add)
                    nc.vector.tensor_mul(dst, y, t2)

            # ---- conv1 (bf16): A' = (W1*g0w) . x ----
            # Stored plane order [owned0, owned1, haloL, haloR]; LOC maps
            # stored idx -> local x position in padded h1. Owned planes run
            # first so GN stats + AR1 launch while halo conv1 still runs.
            LOC = (1, 2, 0, 3)
            ap5 = aprime.rearrange("p (s y z w) -> p s y z w",
                                   s=NPL, y=16, z=16, w=16)
            sta = stile([128, 16, 6], "sta")

            def conv1_plane(sj, with_stats):
                for n in range(8):
                    pt = ps.tile([128, 512], F32, tag="ps", name=f"c1_{sj}_{n}")
                    nc.tensor.matmul(
                        pt,
                        w1_sb[32 * sj:32 * sj + 32, :],
                        x_sb[32 * sj:32 * sj + 32, bass.ts(n, 512)],
                        start=True, stop=True, tile_position=(32 * sj, 0))
                    blk = bass.ts(sj * 8 + n, 512)
                    nc.scalar.copy(out=aprime[:, blk], in_=pt)
                    if with_stats:
                        nc.vector.bn_stats(out=sta[:, sj * 8 + n, :],
                                           in_=aprime[:, blk])

            conv1_plane(0, True)
            conv1_plane(1, True)

            # ---- x stats (owned planes = partitions 0:64, bf16 input) ----
            stx = stile([128, 8, 6], "stx")
            for c in range(8):
                nc.vector.bn_stats(out=stx[0:64, c, :],
                                   in_=x_sb[0:64, bass.ts(c, 512)])
            mvx = stile([128, 2], "mvx")
            nc.vector.bn_aggr(out=mvx[0:64, :], in_=stx[0:64])

            mva = stile([128, 2], "mva")
            nc.vector.bn_aggr(out=mva, in_=sta)

            pk = stile([128, 6], "pk")
            nc.vector.memset(pk, 0.0)
            # col0: SA_o = mean*POS ; col1: SAA_o = (var+mean^2)*POS
            nc.vector.tensor_scalar_mul(out=_col(pk, 0), in0=_col(mva, 0), scalar1=float(POS))
            t_a = sc("t_a")
            nc.vector.tensor_mul(t_a, _col(mva, 0), _col(mva, 0))
            nc.vector.tensor_add(t_a, t_a, _col(mva, 1))
            nc.vector.tensor_scalar_mul(out=_col(pk, 1), in0=t_a, scalar1=float(POS))
            nc.vector.tensor_mul(_col(pk, 2), _col(pp, 0), _col(pk, 0))   # u*SA
            nc.vector.tensor_mul(_col(pk, 3), _col(pp, 1), _col(pk, 0))   # v*SA
            nc.vector.tensor_scalar_mul(out=pk[0:64, 4:5], in0=mvx[0:64, 0:1], scalar1=float(PLANE))
            t_b = sc("t_b")
            nc.vector.tensor_mul(t_b[0:64], mvx[0:64, 0:1], mvx[0:64, 0:1])
            nc.vector.tensor_add(t_b[0:64], t_b[0:64], mvx[0:64, 1:2])
            nc.vector.tensor_scalar_mul(out=pk[0:64, 5:6], in0=t_b[0:64], scalar1=float(PLANE))

            ps_s1 = ps.tile([1, 6], F32, tag="ps", name="ps_s1")
            nc.tensor.matmul(ps_s1, ones, pk, start=True, stop=True)
            d1o = dram.tile([8], F32, name="d1o")
            row1 = stile([1, 6], "row1")
            nc.vector.tensor_copy(out=row1, in_=ps_s1)
            nc.sync.dma_start(out=d1i[0:6], in_=row1)
            nc.gpsimd.collective_compute(
                "AllReduce", mybir.AluOpType.add,
                replica_groups=[list(range(N_CORES))],
                ins=[d1i.opt()], outs=[d1o.opt()])

            # halo-plane conv1 runs during the AR1 mesh
            conv1_plane(2, False)
            conv1_plane(3, False)

            g1 = stile([128, 8], "g1")
            nc.sync.dma_start(out=g1, in_=bass.AP(
                tensor=d1o.tensor, offset=d1o.offset, ap=[[0, 128]] + list(d1o.ap)))

            # ---- scalar chain (replicated on 128 partitions) ----
            def gn_mu_r(g, i_sum, i_ss, nval, tag):
                mu = stile([128, 1], f"mu_{tag}")
                nc.vector.tensor_scalar_mul(out=mu, in0=_col(g, i_sum), scalar1=1.0 / nval)
                ex2 = sc(f"ex2_{tag}")
                nc.vector.tensor_scalar_mul(out=ex2, in0=_col(g, i_ss), scalar1=1.0 / nval)
                var = sc(f"var_{tag}")
                nc.vector.tensor_mul(var, mu, mu)
                nc.vector.tensor_sub(var, ex2, var)
                nc.vector.tensor_scalar_add(out=var, in0=var, scalar1=EPS)
                r = stile([128, 1], f"r_{tag}")
                rsqrt_dve(r, var, tag)
                return mu, r

            # g1 cols: 0 SumSA, 1 SAA, 2 SumU.SA, 3 SumV.SA, 4 Sx, 5 Sxx
            mu0, r0 = gn_mu_r(g1, 4, 5, NX, "0")
            q = stile([128, 1], "q")
            nc.vector.tensor_mul(q, mu0, r0)
            scsa = sc("scsa")                       # Sum(c*SA) = col2 - q*col3
            nc.vector.tensor_mul(scsa, q, _col(g1, 3))
            nc.vector.tensor_sub(scsa, _col(g1, 2), scsa)
            s_c = sc("s_c")                         # Sum(c) = Su - q*Sv
            nc.vector.tensor_mul(s_c, q, _col(pp, 11))
            nc.vector.tensor_sub(s_c, _col(pp, 10), s_c)
            scc = sc("scc")                         # Sum(c^2)
            t_c = sc("t_c")
            nc.vector.tensor_mul(t_c, q, _col(pp, 13))
            nc.vector.tensor_scalar_mul(out=t_c, in0=t_c, scalar1=2.0)
            nc.vector.tensor_sub(scc, _col(pp, 12), t_c)
            nc.vector.tensor_mul(t_c, q, q)
            nc.vector.tensor_mul(t_c, t_c, _col(pp, 14))
            nc.vector.tensor_add(scc, scc, t_c)
            # mu1
            mu1 = stile([128, 1], "mu1")
            nc.vector.tensor_mul(mu1, r0, _col(g1, 0))
            t_d = sc("t_d")
            nc.vector.tensor_scalar_mul(out=t_d, in0=s_c, scalar1=float(P_SP))
            nc.vector.tensor_add(mu1, mu1, t_d)
            nc.vector.tensor_scalar_mul(out=mu1, in0=mu1, scalar1=1.0 / N1)
            # var1 = (r0^2*SAA + 2 r0 scsa + P*scc)/N1 - mu1^2
            v1 = sc("v1")
            nc.vector.tensor_mul(v1, r0, r0)
            nc.vector.tensor_mul(v1, v1, _col(g1, 1))
            t_e = sc("t_e")
            nc.vector.tensor_mul(t_e, r0, scsa)
            nc.vector.tensor_scalar_mul(out=t_e, in0=t_e, scalar1=2.0)
            nc.vector.tensor_add(v1, v1, t_e)
            nc.vector.tensor_scalar_mul(out=t_e, in0=scc, scalar1=float(P_SP))
            nc.vector.tensor_add(v1, v1, t_e)
            nc.vector.tensor_scalar_mul(out=v1, in0=v1, scalar1=1.0 / N1)
            nc.vector.tensor_mul(t_e, mu1, mu1)
            nc.vector.tensor_sub(v1, v1, t_e)
            nc.vector.tensor_scalar_add(out=v1, in0=v1, scalar1=EPS)
            r1 = stile([128, 1], "r1")
            rsqrt_dve(r1, v1, "1")
            al1 = stile([128, 1], "al1")
            nc.vector.tensor_mul(al1, r0, r1)
            nc.vector.tensor_mul(al1, al1, _col(pp, 2))
            be1 = stile([128, 1], "be1")
            nc.vector.tensor_mul(be1, q, _col(pp, 1))        # q*v
            nc.vector.tensor_sub(be1, _col(pp, 0), be1)      # c = u - q*v
            nc.vector.tensor_sub(be1, be1, mu1)              # c - mu1
            nc.vector.tensor_mul(be1, be1, r1)
            nc.vector.tensor_mul(be1, be1, _col(pp, 2))
            nc.vector.tensor_add(be1, be1, _col(pp, 3))
            # halo-edge masks folded into gelu scale/bias: gelu(0*x+0) == 0
            al1L = stile([128, 1], "al1L")
            be1L = stile([128, 1], "be1L")
            al1R = stile([128, 1], "al1R")
            be1R = stile([128, 1], "be1R")
            nc.vector.tensor_mul(al1L, al1, _col(pp, 8))
            nc.vector.tensor_mul(be1L, be1, _col(pp, 8))
            nc.vector.tensor_mul(al1R, al1, _col(pp, 9))
            nc.vector.tensor_mul(be1R, be1, _col(pp, 9))

            # vector finishes the remaining h1 plane zeroing
            nc.vector.memset(h1pl[:, 2, :], 0.0)
            nc.vector.memset(h1pl[:, 3, :], 0.0)

            # ---- h1 = gelu(alpha1*A' + beta1) in half-planes ----
            # local plane order (0=haloL,1,2,3=haloR); conv2 bank b needs
            # y rows [2b, 2b+4) of local planes 0..2, so after the three
            # A-halves bank 0..2 can start while the rest gelu.
            SB = {0: (al1L, be1L), 1: (al1, be1), 2: (al1, be1),
                  3: (al1R, be1R)}
            halves = [(0, 0), (1, 0), (2, 0), (0, 1), (1, 1), (2, 1),
                      (3, 0), (3, 1)]
            INV = (2, 0, 1, 3)   # local plane -> stored plane
            for (lj, hh) in halves:
                sj = INV[lj]
                alx, bex = SB[lj]
                nc.scalar.activation(
                    out=h1f5[:, lj, 1 + 8 * hh:9 + 8 * hh, 1:17, 1:17],
                    in_=ap5[:, sj, 8 * hh:8 * hh + 8],
                    func=AF.Gelu, bias=bex, scale=alx)

            # ---- conv2: 3^4, 81 taps, accumulate in PSUM ----
            h1r5 = h1f5
            w2r = w2_sb
            sth = stile([128, 16, 6], "sth")
            for j in range(2):
                for b in range(8):
                    pt = ps.tile([128, 512], F32, tag="ps", name=f"c2_{j}_{b}")
                    t = 0
                    for dx in range(3):
                        for dy in range(3):
                            for dz in range(3):
                                for dw in range(3):
                                    mov = h1r5[:, j + dx,
                                               2 * b + dy:2 * b + dy + 2,
                                               dz:dz + 16, dw:dw + 16]
                                    nc.tensor.matmul(pt, w2r[:, bass.ts(t, 128)],
                                                     mov,
                                                     start=(t == 0), stop=(t == 80))
                                    t += 1
                    blk = bass.ts(j * 8 + b, 512)
                    nc.scalar.copy(out=h2[:, blk], in_=pt)
                    nc.vector.bn_stats(out=sth[:, j * 8 + b, :],
                                       in_=h2[:, blk])

            mvh = stile([128, 2], "mvh")
            nc.vector.bn_aggr(out=mvh, in_=sth)
            pk2 = stile([128, 2], "pk2")
            nc.vector.tensor_scalar_mul(out=_col(pk2, 0), in0=_col(mvh, 0), scalar1=float(POS))
            t_f = sc("t_f")
            nc.vector.tensor_mul(t_f, _col(mvh, 0), _col(mvh, 0))
            nc.vector.tensor_add(t_f, t_f, _col(mvh, 1))
            nc.vector.tensor_scalar_mul(out=_col(pk2, 1), in0=t_f, scalar1=float(POS))
            ps_s2 = ps.tile([1, 2], F32, tag="ps", name="ps_s2")
            nc.tensor.matmul(ps_s2, ones, pk2, start=True, stop=True)
            d2o = dram.tile([8], F32, name="d2o")
            row2 = stile([1, 2], "row2")
            nc.vector.tensor_copy(out=row2, in_=ps_s2)
            nc.sync.dma_start(out=d2i[0:2], in_=row2)
            nc.gpsimd.collective_compute(
                "AllReduce", mybir.AluOpType.add,
                replica_groups=[list(range(N_CORES))],
                ins=[d2i.opt()], outs=[d2o.opt()])
            g2 = stile([128, 8], "g2")
            nc.sync.dma_start(out=g2, in_=bass.AP(
                tensor=d2o.tensor, offset=d2o.offset, ap=[[0, 128]] + list(d2o.ap)))

            mu2, r2 = gn_mu_r(g2, 0, 1, N1, "2")
            al2 = stile([128, 1], "al2")
            nc.vector.tensor_mul(al2, r2, _col(pp, 4))
            be2 = stile([128, 1], "be2")
            nc.vector.tensor_mul(be2, mu2, al2)
            nc.vector.tensor_sub(be2, _col(pp, 5), be2)

            # ---- gelu(GN2) one-shot; accum_out is the SE partial sum ----
            m_col = stile([128, 1], "m_col")
            nc.scalar.activation(out=h2b, in_=h2,
                                 func=AF.Gelu, bias=be2, scale=al2,
                                 accum_out=m_col)
            # transpose [128,1] -> [1,128] via identity matmul: a
            # partition-strided SBUF->DRAM DMA does 128 scattered 4B reads
            # (~10us!) and stalls the AR3 trigger; a [1,128] row is one
            # contiguous burst.
            ps_t = ps.tile([1, 128], F32, tag="ps", name="ps_t")
            nc.tensor.matmul(ps_t, m_col, id_sb, start=True, stop=True)
            m_row = stile([1, 128], "m_row")
            nc.vector.tensor_copy(out=m_row, in_=ps_t)
            d3i = dram.tile([128], F32, name="d3i")
            d3o = dram.tile([128], F32, name="d3o")
            nc.sync.dma_start(out=d3i, in_=m_row)
            nc.gpsimd.collective_compute(
                "AllReduce", mybir.AluOpType.add,
                replica_groups=[list(range(N_CORES))],
                ins=[d3i.opt()], outs=[d3o.opt()])
            m_sb = stile([128, 1], "m_sb")
            nc.sync.dma_start(out=m_sb, in_=d3o)

            # ---- SE MLP (tiny, replicated on every core) ----
            m_mean = stile([128, 1], "m_mean")
            nc.vector.tensor_scalar_mul(out=m_mean, in0=m_sb, scalar1=1.0 / P_SP)
            ps_se1 = ps.tile([8, 1], F32, tag="ps", name="ps_se1")
            nc.tensor.matmul(ps_se1, pp[:, 16:24], m_mean, start=True, stop=True)
            y1g = stile([8, 1], "y1g")
            nc.scalar.activation(out=y1g, in_=ps_se1, func=AF.Gelu)
            ps_se2 = ps.tile([128, 1], F32, tag="ps", name="ps_se2")
            nc.tensor.matmul(ps_se2, pp[0:8, 56:184], y1g, start=True, stop=True)
            s_sb = stile([128, 1], "s_sb")
            nc.scalar.activation(out=s_sb, in_=ps_se2, func=AF.Sigmoid)
            w3s = small.tile([128, 32], BF16, name="w3s")
            nc.vector.tensor_scalar_mul(out=w3s, in0=pp[:, 24:56], scalar1=s_sb)

            # ---- conv3 (bf16) + stats; y3 shares the h1 slot ----
            y3 = big.tile([CIN, POS], BF16, name="y3", tag="h1slot")
            st3 = stile([32, 16, 6], "st3")
            for n in range(16):
                pt3 = ps.tile([32, 512], F32, tag="ps", name=f"c3_{n}")
                nc.tensor.matmul(pt3, w3s, h2b[:, bass.ts(n, 512)],
                                 start=True, stop=True)
                if n % 2 == 0:
                    nc.vector.tensor_copy(out=y3[:, bass.ts(n, 512)], in_=pt3)
                else:
                    nc.scalar.copy(out=y3[:, bass.ts(n, 512)], in_=pt3)
                nc.vector.bn_stats(out=st3[:, n, :],
                                   in_=y3[:, bass.ts(n, 512)])
            mv3 = stile([32, 2], "mv3")
            nc.vector.bn_aggr(out=mv3, in_=st3)
            pk3 = stile([128, 2], "pk3")
            nc.vector.memset(pk3, 0.0)
            nc.vector.tensor_scalar_mul(out=pk3[0:32, 0:1], in0=mv3[:, 0:1], scalar1=float(POS))
            t_g = sc("t_g")
            nc.vector.tensor_mul(t_g[0:32], mv3[:, 0:1], mv3[:, 0:1])
            nc.vector.tensor_add(t_g[0:32], t_g[0:32], mv3[:, 1:2])
            nc.vector.tensor_scalar_mul(out=pk3[0:32, 1:2], in0=t_g[0:32], scalar1=float(POS))
            ps_s3 = ps.tile([1, 2], F32, tag="ps", name="ps_s3")
            nc.tensor.matmul(ps_s3, ones, pk3, start=True, stop=True)
            d4o = dram.tile([8], F32, name="d4o")
            row3 = stile([1, 2], "row3")
            nc.vector.tensor_copy(out=row3, in_=ps_s3)
            nc.sync.dma_start(out=d4i[0:2], in_=row3)
            nc.gpsimd.collective_compute(
                "AllReduce", mybir.AluOpType.add,
                replica_groups=[list(range(N_CORES))],
                ins=[d4i.opt()], outs=[d4o.opt()])
            g4 = stile([128, 8], "g4")
            nc.sync.dma_start(out=g4, in_=bass.AP(
                tensor=d4o.tensor, offset=d4o.offset, ap=[[0, 128]] + list(d4o.ap)))

            mu3, r3 = gn_mu_r(g4, 0, 1, N3, "3")
            al3 = stile([128, 1], "al3")
            nc.vector.tensor_mul(al3, r3, _col(pp, 6))
            be3 = stile([128, 1], "be3")
            nc.vector.tensor_mul(be3, mu3, al3)
            nc.vector.tensor_sub(be3, _col(pp, 7), be3)

            # final affine in 4 chunks across two engines; each chunk's
            # store DMA starts as soon as that chunk is done. The f32
            # result reuses aprime's SBUF slot.
            y3f = big.tile([CIN, POS], F32, name="y3f", tag="apslot")
            qn = POS // 4
            for qq in range(4):
                # gpsimd runs this op ~3x slower than DVE; give it one chunk
                eng = nc.vector if qq != 1 else nc.gpsimd
                blk = slice(qq * qn, (qq + 1) * qn)
                eng.tensor_scalar(out=y3f[:, blk], in0=y3[:, blk],
                                  scalar1=al3[0:32], scalar2=be3[0:32],
                                  op0=mybir.AluOpType.mult,
                                  op1=mybir.AluOpType.add)
                nc.sync.dma_start(out=out_d[:, blk], in_=y3f[:, blk])

    nc.compile()
    return nc


def _host_prep(inputs):
    x = np.asarray(inputs['x'], np.float32).reshape(CIN, S, S, S, S)
    g0w = np.asarray(inputs['g0_w'], np.float32)
    g0b = np.asarray(inputs['g0_b'], np.float32)
    W1 = np.asarray(inputs['w1'], np.float32).reshape(HID, CIN)
    gn1w = np.asarray(inputs['gn1_w'], np.float32)
    gn1b = np.asarray(inputs['gn1_b'], np.float32)
    w2 = np.asarray(inputs['w2'], np.float32).reshape(HID, HID, 3, 3, 3, 3)
    gn2w = np.asarray(inputs['gn2_w'], np.float32)
    gn2b = np.asarray(inputs['gn2_b'], np.float32)
    se1 = np.asarray(inputs['se_w1'], np.float32)   # [8,128]
    se2 = np.asarray(inputs['se_w2'], np.float32)   # [128,8]
    W3 = np.asarray(inputs['w3'], np.float32).reshape(CIN, HID)
    gn3w = np.asarray(inputs['gn3_w'], np.float32)
    gn3b = np.asarray(inputs['gn3_b'], np.float32)

    w1fold = W1 * g0w[None, :]
    w1rep = np.zeros((128, 128), np.float32)
    for j in range(4):
        w1rep[32 * j:32 * j + 32, :] = w1fold.T
    w1rep = w1rep.astype(ml_dtypes.bfloat16)
    u = W1 @ g0b
    v = W1 @ g0w
    w2t = np.ascontiguousarray(
        w2.transpose(1, 2, 3, 4, 5, 0).reshape(HID, 81 * HID)).astype(
            ml_dtypes.bfloat16)

    params = np.zeros((128, 192), np.float32)
    params[:, 0] = u
    params[:, 1] = v
    params[:, 2] = gn1w
    params[:, 3] = gn1b
    params[:, 4] = gn2w
    params[:, 5] = gn2b
    params[0:32, 6] = gn3w
    params[0:32, 7] = gn3b
    params[:, 10] = u.sum()
    params[:, 11] = v.sum()
    params[:, 12] = (u * u).sum()
    params[:, 13] = (u * v).sum()
    params[:, 14] = (v * v).sum()
    params[:, 16:24] = se1.T
    params[:, 24:56] = W3.T
    params[0:8, 56:184] = se2.T

    xp = np.zeros((CIN, S + 2, S, S, S), np.float32)
    xp[:, 1:S + 1] = x

    in_maps = []
    for k in range(N_CORES):
        p = params.copy()
        p[:, 8] = 0.0 if k == 0 else 1.0
        p[:, 9] = 0.0 if k == N_CORES - 1 else 1.0
        # stored plane order: [owned0, owned1, haloL, haloR]
        idx = [2 * k + 1, 2 * k + 2, 2 * k, 2 * k + 3]
        shard = np.ascontiguousarray(
            xp[:, idx].transpose(1, 0, 2, 3, 4).reshape(128, PLANE)).astype(
                ml_dtypes.bfloat16)
        in_maps.append({"xs": shard, "w1rep": w1rep, "w2t": w2t, "params": p,
                        "ident": np.eye(128, dtype=np.float32)})
    return in_maps


def kernel(**inputs):
    if "nc" not in _cache:
        _cache["nc"] = build_program()
    nc = _cache["nc"]
    in_maps = _host_prep(inputs)
    res = run_bass_kernel_spmd(nc, in_maps, core_ids=list(range(N_CORES)))
    out = np.empty((1, CIN, S, S, S, S), np.float32)
    for k in range(N_CORES):
        out[0, :, 2 * k:2 * k + 2] = res.results[k]["out"].reshape(CIN, 2, S, S, S)
    return out


def run_traced(inputs):
    """Like kernel() but with NTFF tracing; returns (out, BassKernelResults)."""
    if "nc" not in _cache:
        _cache["nc"] = build_program()
    nc = _cache["nc"]
    in_maps = _host_prep(inputs)
    res = run_bass_kernel_spmd(nc, in_maps, core_ids=list(range(N_CORES)),
                               trace=True)
    out = np.empty((1, CIN, S, S, S, S), np.float32)
    for k in range(N_CORES):
        out[0, :, 2 * k:2 * k + 2] = res.results[k]["out"].reshape(CIN, 2, S, S, S)
    return out, res


# revision 21
# speedup vs baseline: 1.0391x; 1.0333x over previous
"""MBConv (4D spatial, 16^4) on 8 TRN2 NeuronCores.

Sharding: spatial-parallel over the first spatial dim X (16 planes ->
2 owned planes per core + 1 halo plane each side, shipped from host).

Math (all on device except weight-only constant folding on host):
  GN0+conv1+GN1 folded: A' = (W1 * g0_w) . x computed once; the two
  global groupnorms reduce to 6 scalars in ONE AllReduce:
    [Sum(A'), Sum(A'^2), Sum(u*SA), Sum(v*SA), Sum(x), Sum(x^2)]
  with u = W1.g0_b, v = W1.g0_w (host constants); then
  h1 = gelu(alpha1 * A' + beta1) per hidden channel.
  conv2 = 81 accumulating PE matmuls per PSUM bank over a zero-padded
  [128ch, 4planes, 18,18,18] SBUF tile (bf16).
  GN2 -> AllReduce(2 scalars); gelu fused with SE mean via accum_out.
  SE mean -> AllReduce(128); SE MLP on-device; scale folded into w3.
  conv3 (bf16); GN3 -> AllReduce(2 scalars); affine; DMA out.

v2 perf structure:
  - warmup AllReduce at t=0 absorbs CC-engine startup + core skew
  - x/conv1/conv3 in bf16 (fp32r matmuls run ~2x slow on HW)
  - conv1 owned planes first; AR1 launches while halo conv1 runs
  - rsqrt via int bit-trick + 2 Newton steps on DVE: the scalar engine
    keeps the Gelu table loaded -> no ACT_TABLE_LOAD on critical path
  - halo masks folded into gelu scale/bias (gelu(0*x+0) == 0)
  - gelu h1 in half-planes ordered to unblock conv2 bank 0 early
  - gelu h2 one-shot [128,8192] with accum_out = SE partial mean
"""

import sys
sys.path.insert(0, '/opt/trn_rl_repo')

import numpy as np
import ml_dtypes

import concourse.bass as bass
import concourse.bacc as bacc
import concourse.tile as tile
import concourse.mybir as mybir
from concourse.bass_utils import run_bass_kernel_spmd

F32 = mybir.dt.float32
I32 = mybir.dt.int32
BF16 = mybir.dt.bfloat16
AF = mybir.ActivationFunctionType
ALU = mybir.AluOpType

N_CORES = 8
S = 16
CIN = 32
HID = 128
EPS = 1e-5
PLANE = S * S * S            # 4096 positions per x-plane
PPAD = 18 * 18 * 18          # padded plane (z/y/w pad 1)
NPL = 4                      # stored planes per core (2 owned + 2 halo)
POS = 2 * PLANE              # owned positions per core
P_SP = S ** 4                # 65536 global spatial positions
NX = CIN * P_SP
N1 = HID * P_SP
N3 = CIN * P_SP

_cache = {}


def _col(t, i):
    return t[:, i:i + 1]


def build_program(trace_scopes=False):
    nc = bacc.Bacc("TRN2", target_bir_lowering=False, debug=False,
                   enable_asserts=False, num_devices=N_CORES)

    xs_d = nc.dram_tensor("xs", [128, PLANE], BF16, kind="ExternalInput").ap()
    w1_d = nc.dram_tensor("w1rep", [128, 128], BF16, kind="ExternalInput").ap()
    w2_d = nc.dram_tensor("w2t", [128, 81 * 128], BF16, kind="ExternalInput").ap()
    pp_d = nc.dram_tensor("params", [128, 192], F32, kind="ExternalInput").ap()
    id_d = nc.dram_tensor("ident", [128, 128], F32, kind="ExternalInput").ap()
    out_d = nc.dram_tensor("out", [CIN, POS], F32, kind="ExternalOutput").ap()

    with tile.TileContext(nc) as tc:
        with tc.tile_pool(name="big", bufs=1) as big, \
             tc.tile_pool(name="small", bufs=1) as small, \
             tc.tile_pool(name="scr", bufs=48) as scr, \
             tc.tile_pool(name="ps", bufs=8, space="PSUM") as ps, \
             tc.tile_pool(name="dram", bufs=1, space="DRAM") as dram:

            def stile(shape, name, pool=None, dtype=F32):
                return (pool or small).tile(shape, dtype, name=name)

            def sc(name, dtype=F32):
                return scr.tile([128, 1], dtype, tag="scr", name=name)

            # ---- persistent SBUF tensors ----
            x_sb = big.tile([128, PLANE], BF16, name="x_sb")
            w1_sb = big.tile([128, 128], BF16, name="w1_sb")
            w2_sb = big.tile([128, 81 * 128], BF16, name="w2_sb")
            pp = big.tile([128, 192], F32, name="pp")
            h1 = big.tile([128, NPL * PPAD], BF16, name="h1", tag="h1slot")
            h2 = big.tile([128, 2 * PLANE], F32, name="h2")
            h2b = big.tile([128, 2 * PLANE], BF16, name="h2b")
            # aprime (conv1 staging) later reused for the final f32 output
            aprime = big.tile([128, NPL * PLANE], BF16, name="aprime",
                              tag="apslot")

            d1i = dram.tile([8], F32, name="d1i")
            d2i = dram.tile([8], F32, name="d2i")
            d4i = dram.tile([8], F32, name="d4i")
            zrow = small.tile([1, 8], F32, name="zrow")
            nc.vector.memset(zrow, 0.0)

            # weights first (conv1's first matmul needs w1 + x chunk 0),
            # then x owned planes, then the rest
            nc.sync.dma_start(out=w1_sb, in_=w1_d)
            for sj in range(2):
                nc.sync.dma_start(out=x_sb[32 * sj:32 * sj + 32, :],
                                  in_=xs_d[32 * sj:32 * sj + 32, :])
            nc.sync.dma_start(out=pp, in_=pp_d)
            for sj in range(2, NPL):
                nc.sync.dma_start(out=x_sb[32 * sj:32 * sj + 32, :],
                                  in_=xs_d[32 * sj:32 * sj + 32, :])
            nc.sync.dma_start(out=w2_sb, in_=w2_d)
            id_sb = big.tile([128, 128], F32, name="id_sb")
            nc.sync.dma_start(out=id_sb, in_=id_d)
            nc.sync.dma_start(out=d1i, in_=zrow)
            nc.sync.dma_start(out=d2i, in_=zrow)
            nc.sync.dma_start(out=d4i, in_=zrow)

            # preload the activation tables used (Copy/Sigmoid/Gelu) while
            # the scalar engine is idle (each first use otherwise costs a
            # 1.3us ACT_TABLE_LOAD, some on the critical path). Gelu last.
            dummy = stile([1, 1], "dummy")
            nc.vector.memset(dummy, 0.0)
            nc.scalar.activation(out=dummy, in_=dummy, func=AF.Sigmoid)
            nc.scalar.copy(out=dummy, in_=dummy)
            nc.scalar.activation(out=dummy, in_=dummy, func=AF.Gelu)

            h1f5 = h1.rearrange("p (j y z w) -> p j y z w", j=NPL, y=18, z=18, w=18)
            h1pl = h1.rearrange("p (j r) -> p j r", j=NPL, r=PPAD)
            # zero h1 (padding must be 0); gelu-consumption order is local
            # planes 0(hL),1,2,3(hR): gpsimd zeroes 0,1; vector zeroes 2,3
            # after its stats work
            nc.gpsimd.memset(h1pl[:, 0, :], 0.0)
            nc.gpsimd.memset(h1pl[:, 1, :], 0.0)

            def interior(j):
                return h1f5[:, j, 1:17, 1:17, 1:17]

            ones = stile([128, 1], "ones")
            nc.vector.memset(ones, 1.0)
            # row-of-ones and a scalar 1 for PE broadcast/transpose of the
            # AllReduce results: a [128,1]<->[128] DMA is partition-strided
            # (128 scattered 4B descriptors, ~3-10us); a [1,N] row is one
            # burst, and the PE outer-product rebuilds the broadcast.
            ones_row = stile([1, 128], "ones_row")
            nc.vector.memset(ones_row, 1.0)
            one_t = stile([1, 1], "one_t")
            nc.vector.memset(one_t, 1.0)

            def bcast_readback(dsrc, n, tag):
                """DRAM row [n] -> SBUF [128, n] via row DMA + PE outer."""
                grow = stile([1, 8], f"grow_{tag}")
                nc.sync.dma_start(out=grow[:, 0:n], in_=dsrc[0:n])
                ps_b = ps.tile([128, 8], F32, tag="ps", name=f"psb_{tag}")
                nc.tensor.matmul(ps_b[:, 0:n], ones_row, grow[:, 0:n],
                                 start=True, stop=True)
                g = stile([128, 8], f"g_{tag}")
                nc.vector.tensor_copy(out=g[:, 0:n], in_=ps_b[:, 0:n])
                return g

            # ---- DVE rsqrt: y = 1/sqrt(v) via bit trick + 2 Newton steps.
            # Keeps the scalar engine's Gelu table resident (no Sqrt table).
            def rsqrt_dve(out, v, tag):
                tb = sc(f"rs_i_{tag}", I32)
                vb = v.bitcast(I32)
                nc.vector.tensor_scalar(out=tb, in0=vb, scalar1=1,
                                        scalar2=None,
                                        op0=ALU.logical_shift_right)
                # magic - (v>>1), via subtract then negate (the fused
                # xor+add int form crashes the walrus backend)
                nc.vector.tensor_scalar(out=tb, in0=tb, scalar1=0x5f3759df,
                                        scalar2=None, op0=ALU.subtract)
                nc.vector.tensor_scalar(out=tb, in0=tb, scalar1=-1,
                                        scalar2=None, op0=ALU.mult)
                y = tb.bitcast(F32)
                h = sc(f"rs_h_{tag}")
                nc.vector.tensor_scalar_mul(out=h, in0=v, scalar1=0.5)
                t2 = sc(f"rs_t_{tag}")
                for it in range(2):
                    dst = out if it == 1 else y
                    nc.vector.tensor_mul(t2, y, y)
                    nc.vector.tensor_mul(t2, t2, h)
                    nc.vector.tensor_scalar(out=t2, in0=t2, scalar1=-1.0,
                                            scalar2=1.5, op0=ALU.mult,
                                            op1=ALU.add)
                    nc.vector.tensor_mul(dst, y, t2)

            # ---- conv1 (bf16): A' = (W1*g0w) . x ----
            # Stored plane order [owned0, owned1, haloL, haloR]; LOC maps
            # stored idx -> local x position in padded h1. Owned planes run
            # first so GN stats + AR1 launch while halo conv1 still runs.
            LOC = (1, 2, 0, 3)
            ap5 = aprime.rearrange("p (s y z w) -> p s y z w",
                                   s=NPL, y=16, z=16, w=16)
            sta = stile([128, 16, 6], "sta")

            def conv1_plane(sj, with_stats):
                for n in range(8):
                    pt = ps.tile([128, 512], F32, tag="ps", name=f"c1_{sj}_{n}")
                    nc.tensor.matmul(
                        pt,
                        w1_sb[32 * sj:32 * sj + 32, :],
                        x_sb[32 * sj:32 * sj + 32, bass.ts(n, 512)],
                        start=True, stop=True, tile_position=(32 * sj, 0))
                    blk = bass.ts(sj * 8 + n, 512)
                    nc.scalar.copy(out=aprime[:, blk], in_=pt)
                    if with_stats:
                        nc.vector.bn_stats(out=sta[:, sj * 8 + n, :],
                                           in_=aprime[:, blk])

            conv1_plane(0, True)
            conv1_plane(1, True)

            # ---- x stats (owned planes = partitions 0:64, bf16 input) ----
            stx = stile([128, 8, 6], "stx")
            for c in range(8):
                nc.vector.bn_stats(out=stx[0:64, c, :],
                                   in_=x_sb[0:64, bass.ts(c, 512)])
            mvx = stile([128, 2], "mvx")
            nc.vector.bn_aggr(out=mvx[0:64, :], in_=stx[0:64])

            mva = stile([128, 2], "mva")
            nc.vector.bn_aggr(out=mva, in_=sta)

            pk = stile([128, 6], "pk")
            nc.vector.memset(pk, 0.0)
            # col0: SA_o = mean*POS ; col1: SAA_o = (var+mean^2)*POS
            nc.vector.tensor_scalar_mul(out=_col(pk, 0), in0=_col(mva, 0), scalar1=float(POS))
            t_a = sc("t_a")
            nc.vector.tensor_mul(t_a, _col(mva, 0), _col(mva, 0))
            nc.vector.tensor_add(t_a, t_a, _col(mva, 1))
            nc.vector.tensor_scalar_mul(out=_col(pk, 1), in0=t_a, scalar1=float(POS))
            nc.vector.tensor_mul(_col(pk, 2), _col(pp, 0), _col(pk, 0))   # u*SA
            nc.vector.tensor_mul(_col(pk, 3), _col(pp, 1), _col(pk, 0))   # v*SA
            nc.vector.tensor_scalar_mul(out=pk[0:64, 4:5], in0=mvx[0:64, 0:1], scalar1=float(PLANE))
            t_b = sc("t_b")
            nc.vector.tensor_mul(t_b[0:64], mvx[0:64, 0:1], mvx[0:64, 0:1])
            nc.vector.tensor_add(t_b[0:64], t_b[0:64], mvx[0:64, 1:2])
            nc.vector.tensor_scalar_mul(out=pk[0:64, 5:6], in0=t_b[0:64], scalar1=float(PLANE))

            ps_s1 = ps.tile([1, 6], F32, tag="ps", name="ps_s1")
            nc.tensor.matmul(ps_s1, ones, pk, start=True, stop=True)
            d1o = dram.tile([8], F32, name="d1o")
            row1 = stile([1, 6], "row1")
            nc.vector.tensor_copy(out=row1, in_=ps_s1)
            nc.sync.dma_start(out=d1i[0:6], in_=row1)
            nc.gpsimd.collective_compute(
                "AllReduce", mybir.AluOpType.add,
                replica_groups=[list(range(N_CORES))],
                ins=[d1i.opt()], outs=[d1o.opt()])

            # halo-plane conv1 runs during the AR1 mesh
            conv1_plane(2, False)
            conv1_plane(3, False)

            g1 = bcast_readback(d1o, 6, "g1")

            # ---- scalar chain (replicated on 128 partitions) ----
            def gn_mu_r(g, i_sum, i_ss, nval, tag):
                mu = stile([128, 1], f"mu_{tag}")
                nc.vector.tensor_scalar_mul(out=mu, in0=_col(g, i_sum), scalar1=1.0 / nval)
                ex2 = sc(f"ex2_{tag}")
                nc.vector.tensor_scalar_mul(out=ex2, in0=_col(g, i_ss), scalar1=1.0 / nval)
                var = sc(f"var_{tag}")
                nc.vector.tensor_mul(var, mu, mu)
                nc.vector.tensor_sub(var, ex2, var)
                nc.vector.tensor_scalar_add(out=var, in0=var, scalar1=EPS)
                r = stile([128, 1], f"r_{tag}")
                rsqrt_dve(r, var, tag)
                return mu, r

            # g1 cols: 0 SumSA, 1 SAA, 2 SumU.SA, 3 SumV.SA, 4 Sx, 5 Sxx
            mu0, r0 = gn_mu_r(g1, 4, 5, NX, "0")
            q = stile([128, 1], "q")
            nc.vector.tensor_mul(q, mu0, r0)
            scsa = sc("scsa")                       # Sum(c*SA) = col2 - q*col3
            nc.vector.tensor_mul(scsa, q, _col(g1, 3))
            nc.vector.tensor_sub(scsa, _col(g1, 2), scsa)
            # s_c / scc depend only on q: compute on gpsimd, concurrent
            # with the vector engine's mu1/v1 work
            s_c = sc("s_c")                         # Sum(c) = Su - q*Sv
            nc.gpsimd.tensor_mul(s_c, q, _col(pp, 11))
            nc.gpsimd.tensor_sub(s_c, _col(pp, 10), s_c)
            scc = sc("scc")                         # Sum(c^2)
            t_c = sc("t_c")
            nc.gpsimd.tensor_mul(t_c, q, _col(pp, 13))
            nc.gpsimd.tensor_scalar_mul(out=t_c, in0=t_c, scalar1=2.0)
            nc.gpsimd.tensor_sub(scc, _col(pp, 12), t_c)
            nc.gpsimd.tensor_mul(t_c, q, q)
            nc.gpsimd.tensor_mul(t_c, t_c, _col(pp, 14))
            nc.gpsimd.tensor_add(scc, scc, t_c)
            # mu1
            mu1 = stile([128, 1], "mu1")
            nc.vector.tensor_mul(mu1, r0, _col(g1, 0))
            t_d = sc("t_d")
            nc.vector.tensor_scalar_mul(out=t_d, in0=s_c, scalar1=float(P_SP))
            nc.vector.tensor_add(mu1, mu1, t_d)
            nc.vector.tensor_scalar_mul(out=mu1, in0=mu1, scalar1=1.0 / N1)
            # var1 = (r0^2*SAA + 2 r0 scsa + P*scc)/N1 - mu1^2
            v1 = sc("v1")
            nc.vector.tensor_mul(v1, r0, r0)
            nc.vector.tensor_mul(v1, v1, _col(g1, 1))
            t_e = sc("t_e")
            nc.vector.tensor_mul(t_e, r0, scsa)
            nc.vector.tensor_scalar_mul(out=t_e, in0=t_e, scalar1=2.0)
            nc.vector.tensor_add(v1, v1, t_e)
            nc.vector.tensor_scalar_mul(out=t_e, in0=scc, scalar1=float(P_SP))
            nc.vector.tensor_add(v1, v1, t_e)
            nc.vector.tensor_scalar_mul(out=v1, in0=v1, scalar1=1.0 / N1)
            nc.vector.tensor_mul(t_e, mu1, mu1)
            nc.vector.tensor_sub(v1, v1, t_e)
            nc.vector.tensor_scalar_add(out=v1, in0=v1, scalar1=EPS)
            r1 = stile([128, 1], "r1")
            rsqrt_dve(r1, v1, "1")
            al1 = stile([128, 1], "al1")
            nc.vector.tensor_mul(al1, r0, r1)
            nc.vector.tensor_mul(al1, al1, _col(pp, 2))
            be1 = stile([128, 1], "be1")
            nc.vector.tensor_mul(be1, q, _col(pp, 1))        # q*v
            nc.vector.tensor_sub(be1, _col(pp, 0), be1)      # c = u - q*v
            nc.vector.tensor_sub(be1, be1, mu1)              # c - mu1
            nc.vector.tensor_mul(be1, be1, r1)
            nc.vector.tensor_mul(be1, be1, _col(pp, 2))
            nc.vector.tensor_add(be1, be1, _col(pp, 3))
            # halo-edge masks folded into gelu scale/bias: gelu(0*x+0) == 0
            al1L = stile([128, 1], "al1L")
            be1L = stile([128, 1], "be1L")
            al1R = stile([128, 1], "al1R")
            be1R = stile([128, 1], "be1R")
            nc.gpsimd.tensor_mul(al1L, al1, _col(pp, 8))
            nc.gpsimd.tensor_mul(be1L, be1, _col(pp, 8))
            nc.vector.tensor_mul(al1R, al1, _col(pp, 9))
            nc.vector.tensor_mul(be1R, be1, _col(pp, 9))

            # vector finishes the remaining h1 plane zeroing
            nc.vector.memset(h1pl[:, 2, :], 0.0)
            nc.vector.memset(h1pl[:, 3, :], 0.0)

            # ---- h1 = gelu(alpha1*A' + beta1) in quarter-planes ----
            # local plane order (0=haloL,1,2,3=haloR); conv2 bank b needs
            # y rows [2b, 2b+4) of local planes 0..2, so after the first
            # three quarter-gelus (~3us) bank 0 can start.
            SB = {0: (al1L, be1L), 1: (al1, be1), 2: (al1, be1),
                  3: (al1R, be1R)}
            quarters = [(lj, qq) for qq in range(4) for lj in range(3)]
            quarters += [(3, qq) for qq in range(4)]
            INV = (2, 0, 1, 3)   # local plane -> stored plane
            for (lj, qq) in quarters:
                sj = INV[lj]
                alx, bex = SB[lj]
                nc.scalar.activation(
                    out=h1f5[:, lj, 1 + 4 * qq:5 + 4 * qq, 1:17, 1:17],
                    in_=ap5[:, sj, 4 * qq:4 * qq + 4],
                    func=AF.Gelu, bias=bex, scale=alx)

            # ---- conv2: 3^4, 81 taps, accumulate in PSUM ----
            h1r5 = h1f5
            w2r = w2_sb
            sth = stile([128, 16, 6], "sth")
            for j in range(2):
                for b in range(8):
                    pt = ps.tile([128, 512], F32, tag="ps", name=f"c2_{j}_{b}")
                    t = 0
                    for dx in range(3):
                        for dy in range(3):
                            for dz in range(3):
                                for dw in range(3):
                                    mov = h1r5[:, j + dx,
                                               2 * b + dy:2 * b + dy + 2,
                                               dz:dz + 16, dw:dw + 16]
                                    nc.tensor.matmul(pt, w2r[:, bass.ts(t, 128)],
                                                     mov,
                                                     start=(t == 0), stop=(t == 80))
                                    t += 1
                    blk = bass.ts(j * 8 + b, 512)
                    nc.scalar.copy(out=h2[:, blk], in_=pt)
                    nc.vector.bn_stats(out=sth[:, j * 8 + b, :],
                                       in_=h2[:, blk])

            mvh = stile([128, 2], "mvh")
            nc.vector.bn_aggr(out=mvh, in_=sth)
            pk2 = stile([128, 2], "pk2")
            nc.vector.tensor_scalar_mul(out=_col(pk2, 0), in0=_col(mvh, 0), scalar1=float(POS))
            t_f = sc("t_f")
            nc.vector.tensor_mul(t_f, _col(mvh, 0), _col(mvh, 0))
            nc.vector.tensor_add(t_f, t_f, _col(mvh, 1))
            nc.vector.tensor_scalar_mul(out=_col(pk2, 1), in0=t_f, scalar1=float(POS))
            ps_s2 = ps.tile([1, 2], F32, tag="ps", name="ps_s2")
            nc.tensor.matmul(ps_s2, ones, pk2, start=True, stop=True)
            d2o = dram.tile([8], F32, name="d2o")
            row2 = stile([1, 2], "row2")
            nc.vector.tensor_copy(out=row2, in_=ps_s2)
            nc.sync.dma_start(out=d2i[0:2], in_=row2)
            nc.gpsimd.collective_compute(
                "AllReduce", mybir.AluOpType.add,
                replica_groups=[list(range(N_CORES))],
                ins=[d2i.opt()], outs=[d2o.opt()])
            g2 = bcast_readback(d2o, 2, "g2")

            mu2, r2 = gn_mu_r(g2, 0, 1, N1, "2")
            al2 = stile([128, 1], "al2")
            nc.vector.tensor_mul(al2, r2, _col(pp, 4))
            be2 = stile([128, 1], "be2")
            nc.vector.tensor_mul(be2, mu2, al2)
            nc.vector.tensor_sub(be2, _col(pp, 5), be2)

            # ---- gelu(GN2) one-shot; accum_out is the SE partial sum ----
            m_col = stile([128, 1], "m_col")
            nc.scalar.activation(out=h2b, in_=h2,
                                 func=AF.Gelu, bias=be2, scale=al2,
                                 accum_out=m_col)
            # transpose [128,1] -> [1,128] via identity matmul: a
            # partition-strided SBUF->DRAM DMA does 128 scattered 4B reads
            # (~10us!) and stalls the AR3 trigger; a [1,128] row is one
            # contiguous burst.
            ps_t = ps.tile([1, 128], F32, tag="ps", name="ps_t")
            nc.tensor.matmul(ps_t, m_col, id_sb, start=True, stop=True)
            m_row = stile([1, 128], "m_row")
            nc.vector.tensor_copy(out=m_row, in_=ps_t)
            d3i = dram.tile([128], F32, name="d3i")
            d3o = dram.tile([128], F32, name="d3o")
            nc.sync.dma_start(out=d3i, in_=m_row)
            nc.gpsimd.collective_compute(
                "AllReduce", mybir.AluOpType.add,
                replica_groups=[list(range(N_CORES))],
                ins=[d3i.opt()], outs=[d3o.opt()])
            # read the 128-float result as one row, transpose back to a
            # column via PE (rhs = [1,1] one)
            m_row2 = stile([1, 128], "m_row2")
            nc.sync.dma_start(out=m_row2, in_=d3o)
            ps_mt = ps.tile([128, 1], F32, tag="ps", name="ps_mt")
            nc.tensor.matmul(ps_mt, m_row2, one_t, start=True, stop=True)
            m_sb = stile([128, 1], "m_sb")
            nc.vector.tensor_copy(out=m_sb, in_=ps_mt)

            # ---- SE MLP (tiny, replicated on every core) ----
            m_mean = stile([128, 1], "m_mean")
            nc.vector.tensor_scalar_mul(out=m_mean, in0=m_sb, scalar1=1.0 / P_SP)
            ps_se1 = ps.tile([8, 1], F32, tag="ps", name="ps_se1")
            nc.tensor.matmul(ps_se1, pp[:, 16:24], m_mean, start=True, stop=True)
            y1g = stile([8, 1], "y1g")
            nc.scalar.activation(out=y1g, in_=ps_se1, func=AF.Gelu)
            ps_se2 = ps.tile([128, 1], F32, tag="ps", name="ps_se2")
            nc.tensor.matmul(ps_se2, pp[0:8, 56:184], y1g, start=True, stop=True)
            s_sb = stile([128, 1], "s_sb")
            nc.scalar.activation(out=s_sb, in_=ps_se2, func=AF.Sigmoid)
            w3s = small.tile([128, 32], BF16, name="w3s")
            nc.vector.tensor_scalar_mul(out=w3s, in0=pp[:, 24:56], scalar1=s_sb)

            # ---- conv3 (bf16) + stats; y3 shares the h1 slot ----
            y3 = big.tile([CIN, POS], BF16, name="y3", tag="h1slot")
            st3 = stile([32, 16, 6], "st3")
            for n in range(16):
                pt3 = ps.tile([32, 512], F32, tag="ps", name=f"c3_{n}")
                nc.tensor.matmul(pt3, w3s, h2b[:, bass.ts(n, 512)],
                                 start=True, stop=True)
                if n % 2 == 0:
                    nc.vector.tensor_copy(out=y3[:, bass.ts(n, 512)], in_=pt3)
                else:
                    nc.scalar.copy(out=y3[:, bass.ts(n, 512)], in_=pt3)
                nc.vector.bn_stats(out=st3[:, n, :],
                                   in_=y3[:, bass.ts(n, 512)])
            mv3 = stile([32, 2], "mv3")
            nc.vector.bn_aggr(out=mv3, in_=st3)
            pk3 = stile([128, 2], "pk3")
            nc.vector.memset(pk3, 0.0)
            nc.vector.tensor_scalar_mul(out=pk3[0:32, 0:1], in0=mv3[:, 0:1], scalar1=float(POS))
            t_g = sc("t_g")
            nc.vector.tensor_mul(t_g[0:32], mv3[:, 0:1], mv3[:, 0:1])
            nc.vector.tensor_add(t_g[0:32], t_g[0:32], mv3[:, 1:2])
            nc.vector.tensor_scalar_mul(out=pk3[0:32, 1:2], in0=t_g[0:32], scalar1=float(POS))
            ps_s3 = ps.tile([1, 2], F32, tag="ps", name="ps_s3")
            nc.tensor.matmul(ps_s3, ones, pk3, start=True, stop=True)
            d4o = dram.tile([8], F32, name="d4o")
            row3 = stile([1, 2], "row3")
            nc.vector.tensor_copy(out=row3, in_=ps_s3)
            nc.sync.dma_start(out=d4i[0:2], in_=row3)
            nc.gpsimd.collective_compute(
                "AllReduce", mybir.AluOpType.add,
                replica_groups=[list(range(N_CORES))],
                ins=[d4i.opt()], outs=[d4o.opt()])
            g4 = bcast_readback(d4o, 2, "g4")

            mu3, r3 = gn_mu_r(g4, 0, 1, N3, "3")
            al3 = stile([128, 1], "al3")
            nc.vector.tensor_mul(al3, r3, _col(pp, 6))
            be3 = stile([128, 1], "be3")
            nc.vector.tensor_mul(be3, mu3, al3)
            nc.vector.tensor_sub(be3, _col(pp, 7), be3)

            # final affine in 4 chunks across two engines; each chunk's
            # store DMA starts as soon as that chunk is done. The f32
            # result reuses aprime's SBUF slot.
            y3f = big.tile([CIN, POS], F32, name="y3f", tag="apslot")
            qn = POS // 4
            for qq in range(4):
                # all on DVE: gpsimd runs this op ~3x slower
                blk = slice(qq * qn, (qq + 1) * qn)
                nc.vector.tensor_scalar(out=y3f[:, blk], in0=y3[:, blk],
                                        scalar1=al3[0:32], scalar2=be3[0:32],
                                        op0=mybir.AluOpType.mult,
                                        op1=mybir.AluOpType.add)
                nc.sync.dma_start(out=out_d[:, blk], in_=y3f[:, blk])

    nc.compile()
    return nc


def _host_prep(inputs):
    x = np.asarray(inputs['x'], np.float32).reshape(CIN, S, S, S, S)
    g0w = np.asarray(inputs['g0_w'], np.float32)
    g0b = np.asarray(inputs['g0_b'], np.float32)
    W1 = np.asarray(inputs['w1'], np.float32).reshape(HID, CIN)
    gn1w = np.asarray(inputs['gn1_w'], np.float32)
    gn1b = np.asarray(inputs['gn1_b'], np.float32)
    w2 = np.asarray(inputs['w2'], np.float32).reshape(HID, HID, 3, 3, 3, 3)
    gn2w = np.asarray(inputs['gn2_w'], np.float32)
    gn2b = np.asarray(inputs['gn2_b'], np.float32)
    se1 = np.asarray(inputs['se_w1'], np.float32)   # [8,128]
    se2 = np.asarray(inputs['se_w2'], np.float32)   # [128,8]
    W3 = np.asarray(inputs['w3'], np.float32).reshape(CIN, HID)
    gn3w = np.asarray(inputs['gn3_w'], np.float32)
    gn3b = np.asarray(inputs['gn3_b'], np.float32)

    w1fold = W1 * g0w[None, :]
    w1rep = np.zeros((128, 128), np.float32)
    for j in range(4):
        w1rep[32 * j:32 * j + 32, :] = w1fold.T
    w1rep = w1rep.astype(ml_dtypes.bfloat16)
    u = W1 @ g0b
    v = W1 @ g0w
    w2t = np.ascontiguousarray(
        w2.transpose(1, 2, 3, 4, 5, 0).reshape(HID, 81 * HID)).astype(
            ml_dtypes.bfloat16)

    params = np.zeros((128, 192), np.float32)
    params[:, 0] = u
    params[:, 1] = v
    params[:, 2] = gn1w
    params[:, 3] = gn1b
    params[:, 4] = gn2w
    params[:, 5] = gn2b
    params[0:32, 6] = gn3w
    params[0:32, 7] = gn3b
    params[:, 10] = u.sum()
    params[:, 11] = v.sum()
    params[:, 12] = (u * u).sum()
    params[:, 13] = (u * v).sum()
    params[:, 14] = (v * v).sum()
    params[:, 16:24] = se1.T
    params[:, 24:56] = W3.T
    params[0:8, 56:184] = se2.T

    xp = np.zeros((CIN, S + 2, S, S, S), np.float32)
    xp[:, 1:S + 1] = x

    in_maps = []
    for k in range(N_CORES):
        p = params.copy()
        p[:, 8] = 0.0 if k == 0 else 1.0
        p[:, 9] = 0.0 if k == N_CORES - 1 else 1.0
        # stored plane order: [owned0, owned1, haloL, haloR]
        idx = [2 * k + 1, 2 * k + 2, 2 * k, 2 * k + 3]
        shard = np.ascontiguousarray(
            xp[:, idx].transpose(1, 0, 2, 3, 4).reshape(128, PLANE)).astype(
                ml_dtypes.bfloat16)
        in_maps.append({"xs": shard, "w1rep": w1rep, "w2t": w2t, "params": p,
                        "ident": np.eye(128, dtype=np.float32)})
    return in_maps


def kernel(**inputs):
    if "nc" not in _cache:
        _cache["nc"] = build_program()
    nc = _cache["nc"]
    in_maps = _host_prep(inputs)
    res = run_bass_kernel_spmd(nc, in_maps, core_ids=list(range(N_CORES)))
    out = np.empty((1, CIN, S, S, S, S), np.float32)
    for k in range(N_CORES):
        out[0, :, 2 * k:2 * k + 2] = res.results[k]["out"].reshape(CIN, 2, S, S, S)
    return out


def run_traced(inputs):
    """Like kernel() but with NTFF tracing; returns (out, BassKernelResults)."""
    if "nc" not in _cache:
        _cache["nc"] = build_program()
    nc = _cache["nc"]
    in_maps = _host_prep(inputs)
    res = run_bass_kernel_spmd(nc, in_maps, core_ids=list(range(N_CORES)),
                               trace=True)
    out = np.empty((1, CIN, S, S, S, S), np.float32)
    for k in range(N_CORES):
        out[0, :, 2 * k:2 * k + 2] = res.results[k]["out"].reshape(CIN, 2, S, S, S)
    return out, res


# revision 28
# speedup vs baseline: 1.0505x; 1.0110x over previous
"""MBConv (4D spatial, 16^4) on 8 TRN2 NeuronCores.

Sharding: spatial-parallel over the first spatial dim X (16 planes ->
2 owned planes per core + 1 halo plane each side, shipped from host).

Math (all on device except weight-only constant folding on host):
  GN0+conv1+GN1 folded: A' = (W1 * g0_w) . x computed once; the two
  global groupnorms reduce to 6 scalars in ONE AllReduce:
    [Sum(A'), Sum(A'^2), Sum(u*SA), Sum(v*SA), Sum(x), Sum(x^2)]
  with u = W1.g0_b, v = W1.g0_w (host constants); then
  h1 = gelu(alpha1 * A' + beta1) per hidden channel.
  conv2 = 81 accumulating PE matmuls per PSUM bank over a zero-padded
  [128ch, 4planes, 18,18,18] SBUF tile (bf16).
  GN2 -> AllReduce(2 scalars); gelu fused with SE mean via accum_out.
  SE mean -> AllReduce(128); SE MLP on-device; scale folded into w3.
  conv3 (bf16); GN3 -> AllReduce(2 scalars); affine; DMA out.

v2 perf structure:
  - warmup AllReduce at t=0 absorbs CC-engine startup + core skew
  - x/conv1/conv3 in bf16 (fp32r matmuls run ~2x slow on HW)
  - conv1 owned planes first; AR1 launches while halo conv1 runs
  - rsqrt via int bit-trick + 2 Newton steps on DVE: the scalar engine
    keeps the Gelu table loaded -> no ACT_TABLE_LOAD on critical path
  - halo masks folded into gelu scale/bias (gelu(0*x+0) == 0)
  - gelu h1 in half-planes ordered to unblock conv2 bank 0 early
  - gelu h2 one-shot [128,8192] with accum_out = SE partial mean
"""

import sys
sys.path.insert(0, '/opt/trn_rl_repo')

import numpy as np
import ml_dtypes

import concourse.bass as bass
import concourse.bacc as bacc
import concourse.tile as tile
import concourse.mybir as mybir
from concourse.bass_utils import run_bass_kernel_spmd

F32 = mybir.dt.float32
I32 = mybir.dt.int32
BF16 = mybir.dt.bfloat16
AF = mybir.ActivationFunctionType
ALU = mybir.AluOpType

N_CORES = 8
S = 16
CIN = 32
HID = 128
EPS = 1e-5
PLANE = S * S * S            # 4096 positions per x-plane
PPAD = 18 * 18 * 18          # padded plane (z/y/w pad 1)
NPL = 4                      # stored planes per core (2 owned + 2 halo)
POS = 2 * PLANE              # owned positions per core
P_SP = S ** 4                # 65536 global spatial positions
NX = CIN * P_SP
N1 = HID * P_SP
N3 = CIN * P_SP

_cache = {}


def _col(t, i):
    return t[:, i:i + 1]


def build_program(trace_scopes=False):
    nc = bacc.Bacc("TRN2", target_bir_lowering=False, debug=False,
                   enable_asserts=False, num_devices=N_CORES)

    xs_d = nc.dram_tensor("xs", [128, PLANE], BF16, kind="ExternalInput").ap()
    w1_d = nc.dram_tensor("w1rep", [128, 128], BF16, kind="ExternalInput").ap()
    w2_d = nc.dram_tensor("w2t", [128, 81 * 128], BF16, kind="ExternalInput").ap()
    pp_d = nc.dram_tensor("params", [128, 192], F32, kind="ExternalInput").ap()
    id_d = nc.dram_tensor("ident", [128, 128], F32, kind="ExternalInput").ap()
    out_d = nc.dram_tensor("out", [CIN, POS], F32, kind="ExternalOutput").ap()

    with tile.TileContext(nc) as tc:
        with tc.tile_pool(name="big", bufs=1) as big, \
             tc.tile_pool(name="small", bufs=1) as small, \
             tc.tile_pool(name="scr", bufs=48) as scr, \
             tc.tile_pool(name="ps", bufs=8, space="PSUM") as ps, \
             tc.tile_pool(name="dram", bufs=1, space="DRAM") as dram:

            def stile(shape, name, pool=None, dtype=F32):
                return (pool or small).tile(shape, dtype, name=name)

            def sc(name, dtype=F32):
                return scr.tile([128, 1], dtype, tag="scr", name=name)

            # ---- persistent SBUF tensors ----
            x_sb = big.tile([128, PLANE], BF16, name="x_sb")
            w1_sb = big.tile([128, 128], BF16, name="w1_sb")
            w2_sb = big.tile([128, 81 * 128], BF16, name="w2_sb")
            pp = big.tile([128, 192], F32, name="pp")
            h1 = big.tile([128, NPL * PPAD], BF16, name="h1", tag="h1slot")
            h2 = big.tile([128, 2 * PLANE], F32, name="h2")
            h2b = big.tile([128, 2 * PLANE], BF16, name="h2b")
            # aprime (conv1 staging) later reused for the final f32 output
            aprime = big.tile([128, NPL * PLANE], BF16, name="aprime",
                              tag="apslot")

            d1i = dram.tile([8], F32, name="d1i")
            d2i = dram.tile([8], F32, name="d2i")
            d4i = dram.tile([8], F32, name="d4i")
            zrow = small.tile([1, 8], F32, name="zrow")
            nc.vector.memset(zrow, 0.0)

            # weights first (conv1's first matmul needs w1 + x chunk 0),
            # then x owned planes, then the rest
            nc.sync.dma_start(out=w1_sb, in_=w1_d)
            for sj in range(2):
                nc.sync.dma_start(out=x_sb[32 * sj:32 * sj + 32, :],
                                  in_=xs_d[32 * sj:32 * sj + 32, :])
            nc.sync.dma_start(out=pp, in_=pp_d)
            for sj in range(2, NPL):
                nc.sync.dma_start(out=x_sb[32 * sj:32 * sj + 32, :],
                                  in_=xs_d[32 * sj:32 * sj + 32, :])
            nc.sync.dma_start(out=w2_sb, in_=w2_d)
            id_sb = big.tile([128, 128], F32, name="id_sb")
            nc.sync.dma_start(out=id_sb, in_=id_d)
            nc.sync.dma_start(out=d1i, in_=zrow)
            nc.sync.dma_start(out=d2i, in_=zrow)
            nc.sync.dma_start(out=d4i, in_=zrow)

            # preload the activation tables used (Copy/Sigmoid/Gelu) while
            # the scalar engine is idle (each first use otherwise costs a
            # 1.3us ACT_TABLE_LOAD, some on the critical path). Gelu last.
            dummy = stile([1, 1], "dummy")
            nc.vector.memset(dummy, 0.0)
            nc.scalar.activation(out=dummy, in_=dummy, func=AF.Sigmoid)
            nc.scalar.copy(out=dummy, in_=dummy)
            nc.scalar.activation(out=dummy, in_=dummy, func=AF.Gelu)

            h1f5 = h1.rearrange("p (j y z w) -> p j y z w", j=NPL, y=18, z=18, w=18)
            h1pl = h1.rearrange("p (j r) -> p j r", j=NPL, r=PPAD)
            # zero h1 (padding must be 0); gelu-consumption order is local
            # planes 0(hL),1,2,3(hR): gpsimd zeroes 0,1; vector zeroes 2,3
            # after its stats work
            nc.gpsimd.memset(h1pl[:, 0, :], 0.0)
            nc.gpsimd.memset(h1pl[:, 1, :], 0.0)

            def interior(j):
                return h1f5[:, j, 1:17, 1:17, 1:17]

            ones = stile([128, 1], "ones")
            nc.vector.memset(ones, 1.0)
            # row-of-ones and a scalar 1 for PE broadcast/transpose of the
            # AllReduce results: a [128,1]<->[128] DMA is partition-strided
            # (128 scattered 4B descriptors, ~3-10us); a [1,N] row is one
            # burst, and the PE outer-product rebuilds the broadcast.
            ones_row = stile([1, 128], "ones_row")
            nc.vector.memset(ones_row, 1.0)
            one_t = stile([1, 1], "one_t")
            nc.vector.memset(one_t, 1.0)

            def bcast_readback(dsrc, n, tag):
                """DRAM row [n] -> SBUF [128, n] via row DMA + PE outer."""
                grow = stile([1, 8], f"grow_{tag}")
                nc.sync.dma_start(out=grow[:, 0:n], in_=dsrc[0:n])
                ps_b = ps.tile([128, 8], F32, tag="ps", name=f"psb_{tag}")
                nc.tensor.matmul(ps_b[:, 0:n], ones_row, grow[:, 0:n],
                                 start=True, stop=True)
                g = stile([128, 8], f"g_{tag}")
                nc.vector.tensor_copy(out=g[:, 0:n], in_=ps_b[:, 0:n])
                return g

            # ---- DVE rsqrt: y = 1/sqrt(v) via bit trick + 2 Newton steps.
            # Keeps the scalar engine's Gelu table resident (no Sqrt table).
            def rsqrt_dve(out, v, tag):
                tb = sc(f"rs_i_{tag}", I32)
                vb = v.bitcast(I32)
                nc.vector.tensor_scalar(out=tb, in0=vb, scalar1=1,
                                        scalar2=None,
                                        op0=ALU.logical_shift_right)
                # magic - (v>>1), via subtract then negate (the fused
                # xor+add int form crashes the walrus backend)
                nc.vector.tensor_scalar(out=tb, in0=tb, scalar1=0x5f3759df,
                                        scalar2=None, op0=ALU.subtract)
                nc.vector.tensor_scalar(out=tb, in0=tb, scalar1=-1,
                                        scalar2=None, op0=ALU.mult)
                y = tb.bitcast(F32)
                h = sc(f"rs_h_{tag}")
                nc.vector.tensor_scalar_mul(out=h, in0=v, scalar1=0.5)
                t2 = sc(f"rs_t_{tag}")
                for it in range(2):
                    dst = out if it == 1 else y
                    nc.vector.tensor_mul(t2, y, y)
                    nc.vector.tensor_mul(t2, t2, h)
                    nc.vector.tensor_scalar(out=t2, in0=t2, scalar1=-1.0,
                                            scalar2=1.5, op0=ALU.mult,
                                            op1=ALU.add)
                    nc.vector.tensor_mul(dst, y, t2)

            # ---- conv1 (bf16): A' = (W1*g0w) . x ----
            # Stored plane order [owned0, owned1, haloL, haloR]; LOC maps
            # stored idx -> local x position in padded h1. Owned planes run
            # first so GN stats + AR1 launch while halo conv1 still runs.
            LOC = (1, 2, 0, 3)
            ap5 = aprime.rearrange("p (s y z w) -> p s y z w",
                                   s=NPL, y=16, z=16, w=16)
            sta = stile([128, 16, 6], "sta")

            def conv1_plane(sj, with_stats):
                for n in range(8):
                    pt = ps.tile([128, 512], F32, tag="ps", name=f"c1_{sj}_{n}")
                    nc.tensor.matmul(
                        pt,
                        w1_sb[32 * sj:32 * sj + 32, :],
                        x_sb[32 * sj:32 * sj + 32, bass.ts(n, 512)],
                        start=True, stop=True, tile_position=(32 * sj, 0))
                    blk = bass.ts(sj * 8 + n, 512)
                    nc.scalar.copy(out=aprime[:, blk], in_=pt)
                    if with_stats:
                        nc.vector.bn_stats(out=sta[:, sj * 8 + n, :],
                                           in_=aprime[:, blk])

            conv1_plane(0, True)
            conv1_plane(1, True)

            # ---- x stats (owned planes = partitions 0:64, bf16 input) ----
            stx = stile([128, 8, 6], "stx")
            for c in range(8):
                nc.vector.bn_stats(out=stx[0:64, c, :],
                                   in_=x_sb[0:64, bass.ts(c, 512)])
            mvx = stile([128, 2], "mvx")
            nc.vector.bn_aggr(out=mvx[0:64, :], in_=stx[0:64])

            mva = stile([128, 2], "mva")
            nc.vector.bn_aggr(out=mva, in_=sta)

            pk = stile([128, 6], "pk")
            nc.vector.memset(pk, 0.0)
            # col0: SA_o = mean*POS ; col1: SAA_o = (var+mean^2)*POS
            nc.vector.tensor_scalar_mul(out=_col(pk, 0), in0=_col(mva, 0), scalar1=float(POS))
            t_a = sc("t_a")
            nc.vector.tensor_mul(t_a, _col(mva, 0), _col(mva, 0))
            nc.vector.tensor_add(t_a, t_a, _col(mva, 1))
            nc.vector.tensor_scalar_mul(out=_col(pk, 1), in0=t_a, scalar1=float(POS))
            nc.vector.tensor_mul(_col(pk, 2), _col(pp, 0), _col(pk, 0))   # u*SA
            nc.vector.tensor_mul(_col(pk, 3), _col(pp, 1), _col(pk, 0))   # v*SA
            nc.vector.tensor_scalar_mul(out=pk[0:64, 4:5], in0=mvx[0:64, 0:1], scalar1=float(PLANE))
            t_b = sc("t_b")
            nc.vector.tensor_mul(t_b[0:64], mvx[0:64, 0:1], mvx[0:64, 0:1])
            nc.vector.tensor_add(t_b[0:64], t_b[0:64], mvx[0:64, 1:2])
            nc.vector.tensor_scalar_mul(out=pk[0:64, 5:6], in0=t_b[0:64], scalar1=float(PLANE))

            ps_s1 = ps.tile([1, 6], F32, tag="ps", name="ps_s1")
            nc.tensor.matmul(ps_s1, ones, pk, start=True, stop=True)
            d1o = dram.tile([8], F32, name="d1o")
            row1 = stile([1, 6], "row1")
            nc.vector.tensor_copy(out=row1, in_=ps_s1)
            nc.sync.dma_start(out=d1i[0:6], in_=row1)
            nc.gpsimd.collective_compute(
                "AllReduce", mybir.AluOpType.add,
                replica_groups=[list(range(N_CORES))],
                ins=[d1i.opt()], outs=[d1o.opt()])

            # halo-plane conv1 runs during the AR1 mesh
            conv1_plane(2, False)
            conv1_plane(3, False)

            g1 = bcast_readback(d1o, 6, "g1")

            # keep the PE clock ramped through the AR1 wait: ~4us of junk
            # matmuls right before conv2 so its first banks run at full
            # p-state (a cold PE runs ~2x slow for its first ~3us)
            jnk = ps.tile([128, 512], F32, tag="ps", name="jnk")
            for n in range(18):
                nc.tensor.matmul(jnk, w1_sb[0:32, :],
                                 x_sb[0:32, bass.ts(n % 8, 512)],
                                 start=True, stop=True)

            # ---- scalar chain (replicated on 128 partitions) ----
            def gn_mu_r(g, i_sum, i_ss, nval, tag):
                mu = stile([128, 1], f"mu_{tag}")
                nc.vector.tensor_scalar_mul(out=mu, in0=_col(g, i_sum), scalar1=1.0 / nval)
                ex2 = sc(f"ex2_{tag}")
                nc.vector.tensor_scalar_mul(out=ex2, in0=_col(g, i_ss), scalar1=1.0 / nval)
                musq = sc(f"msq_{tag}")
                nc.vector.tensor_mul(musq, mu, mu)
                var = sc(f"var_{tag}")
                # var+eps = (ex2 + EPS) - mu^2 in one fused op
                nc.vector.scalar_tensor_tensor(out=var, in0=ex2, scalar=EPS,
                                               in1=musq, op0=ALU.add,
                                               op1=ALU.subtract)
                r = stile([128, 1], f"r_{tag}")
                rsqrt_dve(r, var, tag)
                return mu, r

            # g1 cols: 0 SumSA, 1 SAA, 2 SumU.SA, 3 SumV.SA, 4 Sx, 5 Sxx
            mu0, r0 = gn_mu_r(g1, 4, 5, NX, "0")
            q = stile([128, 1], "q")
            nc.vector.tensor_mul(q, mu0, r0)
            scsa = sc("scsa")                       # Sum(c*SA) = col2 - q*col3
            nc.vector.tensor_mul(scsa, q, _col(g1, 3))
            nc.vector.tensor_sub(scsa, _col(g1, 2), scsa)
            # s_c / scc depend only on q: compute on gpsimd, concurrent
            # with the vector engine's mu1/v1 work
            s_c = sc("s_c")                         # Sum(c) = Su - q*Sv
            nc.gpsimd.tensor_mul(s_c, q, _col(pp, 11))
            nc.gpsimd.tensor_sub(s_c, _col(pp, 10), s_c)
            scc = sc("scc")                         # Sum(c^2)
            t_c = sc("t_c")
            nc.gpsimd.tensor_mul(t_c, q, _col(pp, 13))
            nc.gpsimd.tensor_scalar_mul(out=t_c, in0=t_c, scalar1=2.0)
            nc.gpsimd.tensor_sub(scc, _col(pp, 12), t_c)
            nc.gpsimd.tensor_mul(t_c, q, q)
            nc.gpsimd.tensor_mul(t_c, t_c, _col(pp, 14))
            nc.gpsimd.tensor_add(scc, scc, t_c)
            # mu1
            mu1 = stile([128, 1], "mu1")
            nc.vector.tensor_mul(mu1, r0, _col(g1, 0))
            t_d = sc("t_d")
            nc.vector.tensor_scalar_mul(out=t_d, in0=s_c, scalar1=float(P_SP))
            nc.vector.tensor_add(mu1, mu1, t_d)
            nc.vector.tensor_scalar_mul(out=mu1, in0=mu1, scalar1=1.0 / N1)
            # var1 = (r0^2*SAA + 2 r0 scsa + P*scc)/N1 - mu1^2
            v1 = sc("v1")
            nc.vector.tensor_mul(v1, r0, r0)
            nc.vector.tensor_mul(v1, v1, _col(g1, 1))
            t_e = sc("t_e")
            nc.vector.tensor_mul(t_e, r0, scsa)
            nc.vector.tensor_scalar_mul(out=t_e, in0=t_e, scalar1=2.0)
            nc.vector.tensor_add(v1, v1, t_e)
            nc.vector.tensor_scalar_mul(out=t_e, in0=scc, scalar1=float(P_SP))
            nc.vector.tensor_add(v1, v1, t_e)
            nc.vector.tensor_scalar_mul(out=v1, in0=v1, scalar1=1.0 / N1)
            nc.vector.tensor_mul(t_e, mu1, mu1)
            nc.vector.tensor_sub(v1, v1, t_e)
            nc.vector.tensor_scalar_add(out=v1, in0=v1, scalar1=EPS)
            r1 = stile([128, 1], "r1")
            rsqrt_dve(r1, v1, "1")
            al1 = stile([128, 1], "al1")
            nc.vector.tensor_mul(al1, r0, r1)
            nc.vector.tensor_mul(al1, al1, _col(pp, 2))
            be1 = stile([128, 1], "be1")
            nc.vector.tensor_mul(be1, q, _col(pp, 1))        # q*v
            nc.vector.tensor_sub(be1, _col(pp, 0), be1)      # c = u - q*v
            nc.vector.tensor_sub(be1, be1, mu1)              # c - mu1
            nc.vector.tensor_mul(be1, be1, r1)
            nc.vector.tensor_mul(be1, be1, _col(pp, 2))
            nc.vector.tensor_add(be1, be1, _col(pp, 3))
            # halo-edge masks folded into gelu scale/bias: gelu(0*x+0) == 0
            al1L = stile([128, 1], "al1L")
            be1L = stile([128, 1], "be1L")
            al1R = stile([128, 1], "al1R")
            be1R = stile([128, 1], "be1R")
            nc.gpsimd.tensor_mul(al1L, al1, _col(pp, 8))
            nc.gpsimd.tensor_mul(be1L, be1, _col(pp, 8))
            nc.vector.tensor_mul(al1R, al1, _col(pp, 9))
            nc.vector.tensor_mul(be1R, be1, _col(pp, 9))

            # vector finishes the remaining h1 plane zeroing
            nc.vector.memset(h1pl[:, 2, :], 0.0)
            nc.vector.memset(h1pl[:, 3, :], 0.0)

            # ---- h1 = gelu(alpha1*A' + beta1) in quarter-planes ----
            # local plane order (0=haloL,1,2,3=haloR); conv2 bank b needs
            # y rows [2b, 2b+4) of local planes 0..2, so after the first
            # three quarter-gelus (~3us) bank 0 can start.
            SB = {0: (al1L, be1L), 1: (al1, be1), 2: (al1, be1),
                  3: (al1R, be1R)}
            quarters = [(lj, qq) for qq in range(4) for lj in range(3)]
            quarters += [(3, qq) for qq in range(4)]
            INV = (2, 0, 1, 3)   # local plane -> stored plane
            for (lj, qq) in quarters:
                sj = INV[lj]
                alx, bex = SB[lj]
                nc.scalar.activation(
                    out=h1f5[:, lj, 1 + 4 * qq:5 + 4 * qq, 1:17, 1:17],
                    in_=ap5[:, sj, 4 * qq:4 * qq + 4],
                    func=AF.Gelu, bias=bex, scale=alx)

            # ---- conv2: 3^4, 81 taps, accumulate in PSUM ----
            h1r5 = h1f5
            w2r = w2_sb
            sth = stile([128, 16, 6], "sth")
            for j in range(2):
                for b in range(8):
                    pt = ps.tile([128, 512], F32, tag="ps", name=f"c2_{j}_{b}")
                    t = 0
                    for dx in range(3):
                        for dy in range(3):
                            for dz in range(3):
                                for dw in range(3):
                                    mov = h1r5[:, j + dx,
                                               2 * b + dy:2 * b + dy + 2,
                                               dz:dz + 16, dw:dw + 16]
                                    nc.tensor.matmul(pt, w2r[:, bass.ts(t, 128)],
                                                     mov,
                                                     start=(t == 0), stop=(t == 80))
                                    t += 1
                    blk = bass.ts(j * 8 + b, 512)
                    nc.scalar.copy(out=h2[:, blk], in_=pt)
                    nc.vector.bn_stats(out=sth[:, j * 8 + b, :],
                                       in_=h2[:, blk])

            mvh = stile([128, 2], "mvh")
            nc.vector.bn_aggr(out=mvh, in_=sth)
            pk2 = stile([128, 2], "pk2")
            nc.vector.tensor_scalar_mul(out=_col(pk2, 0), in0=_col(mvh, 0), scalar1=float(POS))
            t_f = sc("t_f")
            nc.vector.tensor_mul(t_f, _col(mvh, 0), _col(mvh, 0))
            nc.vector.tensor_add(t_f, t_f, _col(mvh, 1))
            nc.vector.tensor_scalar_mul(out=_col(pk2, 1), in0=t_f, scalar1=float(POS))
            ps_s2 = ps.tile([1, 2], F32, tag="ps", name="ps_s2")
            nc.tensor.matmul(ps_s2, ones, pk2, start=True, stop=True)
            d2o = dram.tile([8], F32, name="d2o")
            row2 = stile([1, 2], "row2")
            nc.vector.tensor_copy(out=row2, in_=ps_s2)
            nc.sync.dma_start(out=d2i[0:2], in_=row2)
            nc.gpsimd.collective_compute(
                "AllReduce", mybir.AluOpType.add,
                replica_groups=[list(range(N_CORES))],
                ins=[d2i.opt()], outs=[d2o.opt()])
            g2 = bcast_readback(d2o, 2, "g2")

            mu2, r2 = gn_mu_r(g2, 0, 1, N1, "2")
            al2 = stile([128, 1], "al2")
            nc.vector.tensor_mul(al2, r2, _col(pp, 4))
            be2 = stile([128, 1], "be2")
            nc.vector.tensor_mul(be2, mu2, al2)
            nc.vector.tensor_sub(be2, _col(pp, 5), be2)

            # ---- gelu(GN2) one-shot; accum_out is the SE partial sum ----
            m_col = stile([128, 1], "m_col")
            nc.scalar.activation(out=h2b, in_=h2,
                                 func=AF.Gelu, bias=be2, scale=al2,
                                 accum_out=m_col)
            # transpose [128,1] -> [1,128] via identity matmul: a
            # partition-strided SBUF->DRAM DMA does 128 scattered 4B reads
            # (~10us!) and stalls the AR3 trigger; a [1,128] row is one
            # contiguous burst.
            ps_t = ps.tile([1, 128], F32, tag="ps", name="ps_t")
            nc.tensor.matmul(ps_t, m_col, id_sb, start=True, stop=True)
            m_row = stile([1, 128], "m_row")
            nc.vector.tensor_copy(out=m_row, in_=ps_t)
            d3i = dram.tile([128], F32, name="d3i")
            d3o = dram.tile([128], F32, name="d3o")
            nc.sync.dma_start(out=d3i, in_=m_row)
            nc.gpsimd.collective_compute(
                "AllReduce", mybir.AluOpType.add,
                replica_groups=[list(range(N_CORES))],
                ins=[d3i.opt()], outs=[d3o.opt()])
            # read the 128-float result as one row, transpose back to a
            # column via PE (rhs = [1,1] one)
            m_row2 = stile([1, 128], "m_row2")
            nc.sync.dma_start(out=m_row2, in_=d3o)
            ps_mt = ps.tile([128, 1], F32, tag="ps", name="ps_mt")
            nc.tensor.matmul(ps_mt, m_row2, one_t, start=True, stop=True)
            m_sb = stile([128, 1], "m_sb")
            nc.vector.tensor_copy(out=m_sb, in_=ps_mt)

            # preload the Sigmoid table while the AR3 mesh runs (scalar
            # table cache holds ~2 entries; this evicts Copy, keeps Gelu)
            nc.scalar.activation(out=dummy, in_=dummy, func=AF.Sigmoid)

            # ---- SE MLP (tiny, replicated on every core) ----
            m_mean = stile([128, 1], "m_mean")
            nc.vector.tensor_scalar_mul(out=m_mean, in0=m_sb, scalar1=1.0 / P_SP)
            ps_se1 = ps.tile([8, 1], F32, tag="ps", name="ps_se1")
            nc.tensor.matmul(ps_se1, pp[:, 16:24], m_mean, start=True, stop=True)
            y1g = stile([8, 1], "y1g")
            nc.scalar.activation(out=y1g, in_=ps_se1, func=AF.Gelu)
            ps_se2 = ps.tile([128, 1], F32, tag="ps", name="ps_se2")
            nc.tensor.matmul(ps_se2, pp[0:8, 56:184], y1g, start=True, stop=True)
            s_sb = stile([128, 1], "s_sb")
            nc.scalar.activation(out=s_sb, in_=ps_se2, func=AF.Sigmoid)
            w3s = small.tile([128, 32], BF16, name="w3s")
            nc.vector.tensor_scalar_mul(out=w3s, in0=pp[:, 24:56], scalar1=s_sb)

            # ---- conv3 (bf16), 3-up packed: blocks n=3g+j land on
            # partition band 32j of PSUM group g -> 6 evictions instead of
            # 16. Band j>=1 of group 5 is zero-padded so the uniform
            # 3072-sample stats stay exact (zeros don't change sums).
            y3p = big.tile([96, 6 * 512], BF16, name="y3p", tag="h1slot")
            nc.vector.memset(y3p[32:64, 5 * 512:6 * 512], 0.0)
            nc.vector.memset(y3p[64:96, 5 * 512:6 * 512], 0.0)
            st3 = stile([96, 6, 6], "st3")
            for g in range(6):
                nj = 3 if g < 5 else 1
                pt3 = ps.tile([96, 512], F32, tag="ps", name=f"c3_{g}")
                for j in range(nj):
                    n = 3 * g + j
                    nc.tensor.matmul(pt3[32 * j:32 * j + 32, :], w3s,
                                     h2b[:, bass.ts(n, 512)],
                                     start=True, stop=True)
                blk = bass.ts(g, 512)
                nc.scalar.copy(out=y3p[0:32 * nj, blk], in_=pt3[0:32 * nj, :])
                nc.vector.bn_stats(out=st3[:, g, :], in_=y3p[0:96, blk])
            mv3 = stile([96, 2], "mv3")
            nc.vector.bn_aggr(out=mv3, in_=st3)
            pk3 = stile([128, 2], "pk3")
            nc.vector.memset(pk3, 0.0)
            NS3 = 6.0 * 512.0
            nc.vector.tensor_scalar_mul(out=pk3[0:96, 0:1], in0=mv3[:, 0:1], scalar1=NS3)
            t_g = sc("t_g")
            nc.vector.tensor_mul(t_g[0:96], mv3[:, 0:1], mv3[:, 0:1])
            nc.vector.tensor_add(t_g[0:96], t_g[0:96], mv3[:, 1:2])
            nc.vector.tensor_scalar_mul(out=pk3[0:96, 1:2], in0=t_g[0:96], scalar1=NS3)
            ps_s3 = ps.tile([1, 2], F32, tag="ps", name="ps_s3")
            nc.tensor.matmul(ps_s3, ones, pk3, start=True, stop=True)
            d4o = dram.tile([8], F32, name="d4o")
            row3 = stile([1, 2], "row3")
            nc.vector.tensor_copy(out=row3, in_=ps_s3)
            nc.sync.dma_start(out=d4i[0:2], in_=row3)
            nc.gpsimd.collective_compute(
                "AllReduce", mybir.AluOpType.add,
                replica_groups=[list(range(N_CORES))],
                ins=[d4i.opt()], outs=[d4o.opt()])
            g4 = bcast_readback(d4o, 2, "g4")

            mu3, r3 = gn_mu_r(g4, 0, 1, N3, "3")
            al3 = stile([128, 1], "al3")
            nc.vector.tensor_mul(al3, r3, _col(pp, 6))
            be3 = stile([128, 1], "be3")
            nc.vector.tensor_mul(be3, mu3, al3)
            nc.vector.tensor_sub(be3, _col(pp, 7), be3)

            # final affine on the packed layout (one DVE op), then three
            # band-unpack DMAs rebuild [32, POS] in DRAM. The f32 result
            # reuses aprime's SBUF slot. pp cols 6/7 hold gn3 w/b
            # replicated per 32-partition band.
            y3f = big.tile([96, 6 * 512], F32, name="y3f", tag="apslot")
            nc.vector.tensor_scalar(out=y3f, in0=y3p[0:96, :],
                                    scalar1=al3[0:96], scalar2=be3[0:96],
                                    op0=mybir.AluOpType.mult,
                                    op1=mybir.AluOpType.add)
            for j in range(3):
                ng = 6 if j == 0 else 5
                sb_ap = y3f[32 * j:32 * j + 32].rearrange(
                    "p (g i) -> p g i", g=6, i=512)[:, 0:ng, :]
                dram_ap = bass.AP(tensor=out_d.tensor,
                                  offset=out_d.offset + 512 * j,
                                  ap=[[POS, 32], [3 * 512, ng], [1, 512]])
                nc.sync.dma_start(out=dram_ap, in_=sb_ap)

    nc.compile()
    return nc


def _host_prep(inputs):
    x = np.asarray(inputs['x'], np.float32).reshape(CIN, S, S, S, S)
    g0w = np.asarray(inputs['g0_w'], np.float32)
    g0b = np.asarray(inputs['g0_b'], np.float32)
    W1 = np.asarray(inputs['w1'], np.float32).reshape(HID, CIN)
    gn1w = np.asarray(inputs['gn1_w'], np.float32)
    gn1b = np.asarray(inputs['gn1_b'], np.float32)
    w2 = np.asarray(inputs['w2'], np.float32).reshape(HID, HID, 3, 3, 3, 3)
    gn2w = np.asarray(inputs['gn2_w'], np.float32)
    gn2b = np.asarray(inputs['gn2_b'], np.float32)
    se1 = np.asarray(inputs['se_w1'], np.float32)   # [8,128]
    se2 = np.asarray(inputs['se_w2'], np.float32)   # [128,8]
    W3 = np.asarray(inputs['w3'], np.float32).reshape(CIN, HID)
    gn3w = np.asarray(inputs['gn3_w'], np.float32)
    gn3b = np.asarray(inputs['gn3_b'], np.float32)

    w1fold = W1 * g0w[None, :]
    w1rep = np.zeros((128, 128), np.float32)
    for j in range(4):
        w1rep[32 * j:32 * j + 32, :] = w1fold.T
    w1rep = w1rep.astype(ml_dtypes.bfloat16)
    u = W1 @ g0b
    v = W1 @ g0w
    w2t = np.ascontiguousarray(
        w2.transpose(1, 2, 3, 4, 5, 0).reshape(HID, 81 * HID)).astype(
            ml_dtypes.bfloat16)

    params = np.zeros((128, 192), np.float32)
    params[:, 0] = u
    params[:, 1] = v
    params[:, 2] = gn1w
    params[:, 3] = gn1b
    params[:, 4] = gn2w
    params[:, 5] = gn2b
    params[0:96, 6] = np.tile(gn3w, 3)
    params[0:96, 7] = np.tile(gn3b, 3)
    params[:, 10] = u.sum()
    params[:, 11] = v.sum()
    params[:, 12] = (u * u).sum()
    params[:, 13] = (u * v).sum()
    params[:, 14] = (v * v).sum()
    params[:, 16:24] = se1.T
    params[:, 24:56] = W3.T
    params[0:8, 56:184] = se2.T

    xp = np.zeros((CIN, S + 2, S, S, S), np.float32)
    xp[:, 1:S + 1] = x

    in_maps = []
    for k in range(N_CORES):
        p = params.copy()
        p[:, 8] = 0.0 if k == 0 else 1.0
        p[:, 9] = 0.0 if k == N_CORES - 1 else 1.0
        # stored plane order: [owned0, owned1, haloL, haloR]
        idx = [2 * k + 1, 2 * k + 2, 2 * k, 2 * k + 3]
        shard = np.ascontiguousarray(
            xp[:, idx].transpose(1, 0, 2, 3, 4).reshape(128, PLANE)).astype(
                ml_dtypes.bfloat16)
        in_maps.append({"xs": shard, "w1rep": w1rep, "w2t": w2t, "params": p,
                        "ident": np.eye(128, dtype=np.float32)})
    return in_maps


def kernel(**inputs):
    if "nc" not in _cache:
        _cache["nc"] = build_program()
    nc = _cache["nc"]
    in_maps = _host_prep(inputs)
    res = run_bass_kernel_spmd(nc, in_maps, core_ids=list(range(N_CORES)))
    out = np.empty((1, CIN, S, S, S, S), np.float32)
    for k in range(N_CORES):
        out[0, :, 2 * k:2 * k + 2] = res.results[k]["out"].reshape(CIN, 2, S, S, S)
    return out


def run_traced(inputs):
    """Like kernel() but with NTFF tracing; returns (out, BassKernelResults)."""
    if "nc" not in _cache:
        _cache["nc"] = build_program()
    nc = _cache["nc"]
    in_maps = _host_prep(inputs)
    res = run_bass_kernel_spmd(nc, in_maps, core_ids=list(range(N_CORES)),
                               trace=True)
    out = np.empty((1, CIN, S, S, S, S), np.float32)
    for k in range(N_CORES):
        out[0, :, 2 * k:2 * k + 2] = res.results[k]["out"].reshape(CIN, 2, S, S, S)
    return out, res


# revision 32
# speedup vs baseline: 1.0561x; 1.0054x over previous
"""MBConv (4D spatial, 16^4) on 8 TRN2 NeuronCores.

Sharding: spatial-parallel over the first spatial dim X (16 planes ->
2 owned planes per core + 1 halo plane each side, shipped from host).

Math (all on device except weight-only constant folding on host):
  GN0+conv1+GN1 folded: A' = (W1 * g0_w) . x computed once; the two
  global groupnorms reduce to 6 scalars in ONE AllReduce:
    [Sum(A'), Sum(A'^2), Sum(u*SA), Sum(v*SA), Sum(x), Sum(x^2)]
  with u = W1.g0_b, v = W1.g0_w (host constants); then
  h1 = gelu(alpha1 * A' + beta1) per hidden channel.
  conv2 = 81 accumulating PE matmuls per PSUM bank over a zero-padded
  [128ch, 4planes, 18,18,18] SBUF tile (bf16).
  GN2 -> AllReduce(2 scalars); gelu fused with SE mean via accum_out.
  SE mean -> AllReduce(128); SE MLP on-device; scale folded into w3.
  conv3 (bf16); GN3 -> AllReduce(2 scalars); affine; DMA out.

v2 perf structure:
  - warmup AllReduce at t=0 absorbs CC-engine startup + core skew
  - x/conv1/conv3 in bf16 (fp32r matmuls run ~2x slow on HW)
  - conv1 owned planes first; AR1 launches while halo conv1 runs
  - rsqrt via int bit-trick + 2 Newton steps on DVE: the scalar engine
    keeps the Gelu table loaded -> no ACT_TABLE_LOAD on critical path
  - halo masks folded into gelu scale/bias (gelu(0*x+0) == 0)
  - gelu h1 in half-planes ordered to unblock conv2 bank 0 early
  - gelu h2 one-shot [128,8192] with accum_out = SE partial mean
"""

import sys
sys.path.insert(0, '/opt/trn_rl_repo')

import numpy as np
import ml_dtypes

import concourse.bass as bass
import concourse.bacc as bacc
import concourse.tile as tile
import concourse.mybir as mybir
from concourse.bass_utils import run_bass_kernel_spmd

F32 = mybir.dt.float32
I32 = mybir.dt.int32
BF16 = mybir.dt.bfloat16
AF = mybir.ActivationFunctionType
ALU = mybir.AluOpType

N_CORES = 8
S = 16
CIN = 32
HID = 128
EPS = 1e-5
PLANE = S * S * S            # 4096 positions per x-plane
PPAD = 18 * 18 * 18          # padded plane (z/y/w pad 1)
NPL = 4                      # stored planes per core (2 owned + 2 halo)
POS = 2 * PLANE              # owned positions per core
P_SP = S ** 4                # 65536 global spatial positions
NX = CIN * P_SP
N1 = HID * P_SP
N3 = CIN * P_SP

_cache = {}


def _col(t, i):
    return t[:, i:i + 1]


def build_program(trace_scopes=False):
    nc = bacc.Bacc("TRN2", target_bir_lowering=False, debug=False,
                   enable_asserts=False, num_devices=N_CORES)

    xs_d = nc.dram_tensor("xs", [128, PLANE], BF16, kind="ExternalInput").ap()
    w1_d = nc.dram_tensor("w1rep", [128, 128], BF16, kind="ExternalInput").ap()
    w2_d = nc.dram_tensor("w2t", [128, 81 * 128], BF16, kind="ExternalInput").ap()
    pp_d = nc.dram_tensor("params", [128, 192], F32, kind="ExternalInput").ap()
    id_d = nc.dram_tensor("ident", [128, 128], F32, kind="ExternalInput").ap()
    out_d = nc.dram_tensor("out", [CIN, POS], F32, kind="ExternalOutput").ap()

    with tile.TileContext(nc) as tc:
        with tc.tile_pool(name="big", bufs=1) as big, \
             tc.tile_pool(name="small", bufs=1) as small, \
             tc.tile_pool(name="scr", bufs=48) as scr, \
             tc.tile_pool(name="ps", bufs=8, space="PSUM") as ps, \
             tc.tile_pool(name="dram", bufs=1, space="DRAM") as dram:

            def stile(shape, name, pool=None, dtype=F32):
                return (pool or small).tile(shape, dtype, name=name)

            def sc(name, dtype=F32):
                return scr.tile([128, 1], dtype, tag="scr", name=name)

            # ---- persistent SBUF tensors ----
            x_sb = big.tile([128, PLANE], BF16, name="x_sb")
            w1_sb = big.tile([128, 128], BF16, name="w1_sb")
            w2_sb = big.tile([128, 81 * 128], BF16, name="w2_sb")
            pp = big.tile([128, 192], F32, name="pp")
            h1 = big.tile([128, NPL * PPAD], BF16, name="h1", tag="h1slot")
            h2 = big.tile([128, 2 * PLANE], F32, name="h2")
            h2b = big.tile([128, 2 * PLANE], BF16, name="h2b")
            # aprime (conv1 staging) later reused for the final f32 output
            aprime = big.tile([128, NPL * PLANE], BF16, name="aprime",
                              tag="apslot")

            d1i = dram.tile([8], F32, name="d1i")
            d2i = dram.tile([8], F32, name="d2i")
            d4i = dram.tile([8], F32, name="d4i")
            zrow = small.tile([1, 8], F32, name="zrow")
            nc.vector.memset(zrow, 0.0)

            # weights first (conv1's first matmul needs w1 + x chunk 0),
            # then x owned planes, then the rest
            nc.sync.dma_start(out=w1_sb, in_=w1_d)
            for sj in range(2):
                nc.sync.dma_start(out=x_sb[32 * sj:32 * sj + 32, :],
                                  in_=xs_d[32 * sj:32 * sj + 32, :])
            nc.sync.dma_start(out=pp, in_=pp_d)
            for sj in range(2, NPL):
                nc.sync.dma_start(out=x_sb[32 * sj:32 * sj + 32, :],
                                  in_=xs_d[32 * sj:32 * sj + 32, :])
            nc.sync.dma_start(out=w2_sb, in_=w2_d)
            id_sb = big.tile([128, 128], F32, name="id_sb")
            nc.sync.dma_start(out=id_sb, in_=id_d)
            nc.sync.dma_start(out=d1i, in_=zrow)
            nc.sync.dma_start(out=d2i, in_=zrow)
            nc.sync.dma_start(out=d4i, in_=zrow)

            # preload the activation tables used (Copy/Sigmoid/Gelu) while
            # the scalar engine is idle (each first use otherwise costs a
            # 1.3us ACT_TABLE_LOAD, some on the critical path). Gelu last.
            dummy = stile([1, 1], "dummy")
            nc.vector.memset(dummy, 0.0)
            nc.scalar.activation(out=dummy, in_=dummy, func=AF.Sigmoid)
            nc.scalar.copy(out=dummy, in_=dummy)
            nc.scalar.activation(out=dummy, in_=dummy, func=AF.Gelu)

            h1f5 = h1.rearrange("p (j y z w) -> p j y z w", j=NPL, y=18, z=18, w=18)
            h1pl = h1.rearrange("p (j r) -> p j r", j=NPL, r=PPAD)
            # zero h1 (padding must be 0); gelu-consumption order is local
            # planes 0(hL),1,2,3(hR): gpsimd zeroes 0,1; vector zeroes 2,3
            # after its stats work
            nc.gpsimd.memset(h1pl[:, 0, :], 0.0)
            nc.gpsimd.memset(h1pl[:, 1, :], 0.0)

            def interior(j):
                return h1f5[:, j, 1:17, 1:17, 1:17]

            ones = stile([128, 1], "ones")
            nc.vector.memset(ones, 1.0)
            # row-of-ones and a scalar 1 for PE broadcast/transpose of the
            # AllReduce results: a [128,1]<->[128] DMA is partition-strided
            # (128 scattered 4B descriptors, ~3-10us); a [1,N] row is one
            # burst, and the PE outer-product rebuilds the broadcast.
            ones_row = stile([1, 128], "ones_row")
            nc.vector.memset(ones_row, 1.0)
            one_t = stile([1, 1], "one_t")
            nc.vector.memset(one_t, 1.0)

            def bcast_readback(dsrc, n, tag):
                """DRAM row [n] -> SBUF [128, n] via row DMA + PE outer."""
                grow = stile([1, 8], f"grow_{tag}")
                nc.sync.dma_start(out=grow[:, 0:n], in_=dsrc[0:n])
                ps_b = ps.tile([128, 8], F32, tag="ps", name=f"psb_{tag}")
                nc.tensor.matmul(ps_b[:, 0:n], ones_row, grow[:, 0:n],
                                 start=True, stop=True)
                g = stile([128, 8], f"g_{tag}")
                nc.vector.tensor_copy(out=g[:, 0:n], in_=ps_b[:, 0:n])
                return g

            # ---- DVE rsqrt: y = 1/sqrt(v) via bit trick + 2 Newton steps.
            # Keeps the scalar engine's Gelu table resident (no Sqrt table).
            def rsqrt_dve(out, v, tag):
                tb = sc(f"rs_i_{tag}", I32)
                vb = v.bitcast(I32)
                nc.vector.tensor_scalar(out=tb, in0=vb, scalar1=1,
                                        scalar2=None,
                                        op0=ALU.logical_shift_right)
                # magic - (v>>1), via subtract then negate (the fused
                # xor+add int form crashes the walrus backend)
                nc.vector.tensor_scalar(out=tb, in0=tb, scalar1=0x5f3759df,
                                        scalar2=None, op0=ALU.subtract)
                nc.vector.tensor_scalar(out=tb, in0=tb, scalar1=-1,
                                        scalar2=None, op0=ALU.mult)
                y = tb.bitcast(F32)
                h = sc(f"rs_h_{tag}")
                nc.vector.tensor_scalar_mul(out=h, in0=v, scalar1=0.5)
                t2 = sc(f"rs_t_{tag}")
                niter = 2 if tag == "0" else 1
                for it in range(niter):
                    dst = out if it == niter - 1 else y
                    nc.vector.tensor_mul(t2, y, y)
                    nc.vector.tensor_mul(t2, t2, h)
                    nc.vector.tensor_scalar(out=t2, in0=t2, scalar1=-1.0,
                                            scalar2=1.5, op0=ALU.mult,
                                            op1=ALU.add)
                    nc.vector.tensor_mul(dst, y, t2)

            # ---- conv1 (bf16): A' = (W1*g0w) . x ----
            # Stored plane order [owned0, owned1, haloL, haloR]; LOC maps
            # stored idx -> local x position in padded h1. Owned planes run
            # first so GN stats + AR1 launch while halo conv1 still runs.
            LOC = (1, 2, 0, 3)
            ap5 = aprime.rearrange("p (s y z w) -> p s y z w",
                                   s=NPL, y=16, z=16, w=16)
            sta = stile([128, 16, 6], "sta")

            def conv1_plane(sj, with_stats):
                for n in range(8):
                    pt = ps.tile([128, 512], F32, tag="ps", name=f"c1_{sj}_{n}")
                    nc.tensor.matmul(
                        pt,
                        w1_sb[32 * sj:32 * sj + 32, :],
                        x_sb[32 * sj:32 * sj + 32, bass.ts(n, 512)],
                        start=True, stop=True, tile_position=(32 * sj, 0))
                    blk = bass.ts(sj * 8 + n, 512)
                    nc.scalar.copy(out=aprime[:, blk], in_=pt)
                    if with_stats:
                        nc.vector.bn_stats(out=sta[:, sj * 8 + n, :],
                                           in_=aprime[:, blk])

            conv1_plane(0, True)
            conv1_plane(1, True)

            # ---- x stats (owned planes = partitions 0:64, bf16 input) ----
            stx = stile([128, 8, 6], "stx")
            for c in range(8):
                nc.vector.bn_stats(out=stx[0:64, c, :],
                                   in_=x_sb[0:64, bass.ts(c, 512)])
            mvx = stile([128, 2], "mvx")
            nc.vector.bn_aggr(out=mvx[0:64, :], in_=stx[0:64])

            mva = stile([128, 2], "mva")
            nc.vector.bn_aggr(out=mva, in_=sta)

            pk = stile([128, 6], "pk")
            nc.vector.memset(pk, 0.0)
            # col0: SA_o = mean*POS ; col1: SAA_o = (var+mean^2)*POS
            nc.vector.tensor_scalar_mul(out=_col(pk, 0), in0=_col(mva, 0), scalar1=float(POS))
            t_a = sc("t_a")
            nc.vector.tensor_mul(t_a, _col(mva, 0), _col(mva, 0))
            nc.vector.tensor_add(t_a, t_a, _col(mva, 1))
            nc.vector.tensor_scalar_mul(out=_col(pk, 1), in0=t_a, scalar1=float(POS))
            nc.vector.tensor_mul(_col(pk, 2), _col(pp, 0), _col(pk, 0))   # u*SA
            nc.vector.tensor_mul(_col(pk, 3), _col(pp, 1), _col(pk, 0))   # v*SA
            nc.vector.tensor_scalar_mul(out=pk[0:64, 4:5], in0=mvx[0:64, 0:1], scalar1=float(PLANE))
            t_b = sc("t_b")
            nc.vector.tensor_mul(t_b[0:64], mvx[0:64, 0:1], mvx[0:64, 0:1])
            nc.vector.tensor_add(t_b[0:64], t_b[0:64], mvx[0:64, 1:2])
            nc.vector.tensor_scalar_mul(out=pk[0:64, 5:6], in0=t_b[0:64], scalar1=float(PLANE))

            ps_s1 = ps.tile([1, 6], F32, tag="ps", name="ps_s1")
            nc.tensor.matmul(ps_s1, ones, pk, start=True, stop=True)
            d1o = dram.tile([8], F32, name="d1o")
            row1 = stile([1, 6], "row1")
            nc.vector.tensor_copy(out=row1, in_=ps_s1)
            nc.sync.dma_start(out=d1i[0:6], in_=row1)
            nc.gpsimd.collective_compute(
                "AllReduce", mybir.AluOpType.add,
                replica_groups=[list(range(N_CORES))],
                ins=[d1i.opt()], outs=[d1o.opt()])

            # halo-plane conv1 runs during the AR1 mesh
            conv1_plane(2, False)
            conv1_plane(3, False)
            # re-touch the Gelu table after the conv1 Copy evictions (the
            # table cache is tiny; this reload runs during the AR1 wait
            # instead of on the gelu-h1 critical path)
            nc.scalar.activation(out=dummy, in_=dummy, func=AF.Gelu)

            g1 = bcast_readback(d1o, 6, "g1")

            # ---- scalar chain (replicated on 128 partitions) ----
            def gn_mu_r(g, i_sum, i_ss, nval, tag):
                mu = stile([128, 1], f"mu_{tag}")
                nc.vector.tensor_scalar_mul(out=mu, in0=_col(g, i_sum), scalar1=1.0 / nval)
                ex2 = sc(f"ex2_{tag}")
                nc.vector.tensor_scalar_mul(out=ex2, in0=_col(g, i_ss), scalar1=1.0 / nval)
                musq = sc(f"msq_{tag}")
                nc.vector.tensor_mul(musq, mu, mu)
                var = sc(f"var_{tag}")
                # var+eps = (ex2 + EPS) - mu^2 in one fused op
                nc.vector.scalar_tensor_tensor(out=var, in0=ex2, scalar=EPS,
                                               in1=musq, op0=ALU.add,
                                               op1=ALU.subtract)
                r = stile([128, 1], f"r_{tag}")
                rsqrt_dve(r, var, tag)
                return mu, r

            # g1 cols: 0 SumSA, 1 SAA, 2 SumU.SA, 3 SumV.SA, 4 Sx, 5 Sxx
            mu0, r0 = gn_mu_r(g1, 4, 5, NX, "0")
            q = stile([128, 1], "q")
            nc.vector.tensor_mul(q, mu0, r0)
            scsa = sc("scsa")                       # Sum(c*SA) = col2 - q*col3
            nc.vector.tensor_mul(scsa, q, _col(g1, 3))
            nc.vector.tensor_sub(scsa, _col(g1, 2), scsa)
            # s_c / scc depend only on q: compute on gpsimd, concurrent
            # with the vector engine's mu1/v1 work
            s_c = sc("s_c")                         # Sum(c) = Su - q*Sv
            nc.gpsimd.tensor_mul(s_c, q, _col(pp, 11))
            nc.gpsimd.tensor_sub(s_c, _col(pp, 10), s_c)
            scc = sc("scc")                         # Sum(c^2)
            t_c = sc("t_c")
            nc.gpsimd.tensor_mul(t_c, q, _col(pp, 13))
            nc.gpsimd.tensor_scalar_mul(out=t_c, in0=t_c, scalar1=2.0)
            nc.gpsimd.tensor_sub(scc, _col(pp, 12), t_c)
            nc.gpsimd.tensor_mul(t_c, q, q)
            nc.gpsimd.tensor_mul(t_c, t_c, _col(pp, 14))
            nc.gpsimd.tensor_add(scc, scc, t_c)
            # mu1
            mu1 = stile([128, 1], "mu1")
            nc.vector.tensor_mul(mu1, r0, _col(g1, 0))
            t_d = sc("t_d")
            nc.vector.tensor_scalar_mul(out=t_d, in0=s_c, scalar1=float(P_SP))
            nc.vector.tensor_add(mu1, mu1, t_d)
            nc.vector.tensor_scalar_mul(out=mu1, in0=mu1, scalar1=1.0 / N1)
            # var1 = (r0^2*SAA + 2 r0 scsa + P*scc)/N1 - mu1^2
            v1 = sc("v1")
            nc.vector.tensor_mul(v1, r0, r0)
            nc.vector.tensor_mul(v1, v1, _col(g1, 1))
            t_e = sc("t_e")
            nc.vector.tensor_mul(t_e, r0, scsa)
            nc.vector.tensor_scalar_mul(out=t_e, in0=t_e, scalar1=2.0)
            nc.vector.tensor_add(v1, v1, t_e)
            nc.vector.tensor_scalar_mul(out=t_e, in0=scc, scalar1=float(P_SP))
            nc.vector.tensor_add(v1, v1, t_e)
            nc.vector.tensor_scalar_mul(out=v1, in0=v1, scalar1=1.0 / N1)
            nc.vector.tensor_mul(t_e, mu1, mu1)
            nc.vector.tensor_sub(v1, v1, t_e)
            nc.vector.tensor_scalar_add(out=v1, in0=v1, scalar1=EPS)
            r1 = stile([128, 1], "r1")
            rsqrt_dve(r1, v1, "1")
            al1 = stile([128, 1], "al1")
            nc.vector.tensor_mul(al1, r0, r1)
            nc.vector.tensor_mul(al1, al1, _col(pp, 2))
            be1 = stile([128, 1], "be1")
            nc.vector.tensor_mul(be1, q, _col(pp, 1))        # q*v
            nc.vector.tensor_sub(be1, _col(pp, 0), be1)      # c = u - q*v
            nc.vector.tensor_sub(be1, be1, mu1)              # c - mu1
            nc.vector.tensor_mul(be1, be1, r1)
            nc.vector.tensor_mul(be1, be1, _col(pp, 2))
            nc.vector.tensor_add(be1, be1, _col(pp, 3))
            # halo-edge masks folded into gelu scale/bias: gelu(0*x+0) == 0
            al1L = stile([128, 1], "al1L")
            be1L = stile([128, 1], "be1L")
            al1R = stile([128, 1], "al1R")
            be1R = stile([128, 1], "be1R")
            nc.gpsimd.tensor_mul(al1L, al1, _col(pp, 8))
            nc.gpsimd.tensor_mul(be1L, be1, _col(pp, 8))
            nc.vector.tensor_mul(al1R, al1, _col(pp, 9))
            nc.vector.tensor_mul(be1R, be1, _col(pp, 9))

            # PE p-state keep-warm: ~2.6us of junk matmuls gated on al1 so
            # they run in the window between the chain and conv2's first
            # bank (a cooled PE runs ~2x slow for its first ~3us)
            al1b = stile([128, 1], "al1b", dtype=BF16)
            nc.vector.tensor_copy(out=al1b, in_=al1)
            jnk = ps.tile([1, 512], F32, tag="ps", name="jnk")
            for n in range(12):
                nc.tensor.matmul(jnk, al1b, x_sb[:, bass.ts(n % 8, 512)],
                                 start=True, stop=True)

            # vector finishes the remaining h1 plane zeroing
            nc.vector.memset(h1pl[:, 2, :], 0.0)
            nc.vector.memset(h1pl[:, 3, :], 0.0)

            # ---- h1 = gelu(alpha1*A' + beta1) in quarter-planes ----
            # local plane order (0=haloL,1,2,3=haloR); conv2 bank b needs
            # y rows [2b, 2b+4) of local planes 0..2, so after the first
            # three quarter-gelus (~3us) bank 0 can start.
            SB = {0: (al1L, be1L), 1: (al1, be1), 2: (al1, be1),
                  3: (al1R, be1R)}
            quarters = [(lj, qq) for qq in range(4) for lj in range(3)]
            quarters += [(3, qq) for qq in range(4)]
            INV = (2, 0, 1, 3)   # local plane -> stored plane
            for (lj, qq) in quarters:
                sj = INV[lj]
                alx, bex = SB[lj]
                nc.scalar.activation(
                    out=h1f5[:, lj, 1 + 4 * qq:5 + 4 * qq, 1:17, 1:17],
                    in_=ap5[:, sj, 4 * qq:4 * qq + 4],
                    func=AF.Gelu, bias=bex, scale=alx)

            # ---- conv2: 3^4, 81 taps, accumulate in PSUM ----
            h1r5 = h1f5
            w2r = w2_sb
            sth = stile([128, 16, 6], "sth")
            for j in range(2):
                for b in range(8):
                    pt = ps.tile([128, 512], F32, tag="ps", name=f"c2_{j}_{b}")
                    t = 0
                    for dx in range(3):
                        for dy in range(3):
                            for dz in range(3):
                                for dw in range(3):
                                    mov = h1r5[:, j + dx,
                                               2 * b + dy:2 * b + dy + 2,
                                               dz:dz + 16, dw:dw + 16]
                                    nc.tensor.matmul(pt, w2r[:, bass.ts(t, 128)],
                                                     mov,
                                                     start=(t == 0), stop=(t == 80))
                                    t += 1
                    blk = bass.ts(j * 8 + b, 512)
                    nc.scalar.copy(out=h2[:, blk], in_=pt)
                    nc.vector.bn_stats(out=sth[:, j * 8 + b, :],
                                       in_=h2[:, blk])

            mvh = stile([128, 2], "mvh")
            nc.vector.bn_aggr(out=mvh, in_=sth)
            pk2 = stile([128, 2], "pk2")
            nc.vector.tensor_scalar_mul(out=_col(pk2, 0), in0=_col(mvh, 0), scalar1=float(POS))
            t_f = sc("t_f")
            nc.vector.tensor_mul(t_f, _col(mvh, 0), _col(mvh, 0))
            nc.vector.tensor_add(t_f, t_f, _col(mvh, 1))
            nc.vector.tensor_scalar_mul(out=_col(pk2, 1), in0=t_f, scalar1=float(POS))
            ps_s2 = ps.tile([1, 2], F32, tag="ps", name="ps_s2")
            nc.tensor.matmul(ps_s2, ones, pk2, start=True, stop=True)
            d2o = dram.tile([8], F32, name="d2o")
            row2 = stile([1, 2], "row2")
            nc.vector.tensor_copy(out=row2, in_=ps_s2)
            nc.sync.dma_start(out=d2i[0:2], in_=row2)
            nc.gpsimd.collective_compute(
                "AllReduce", mybir.AluOpType.add,
                replica_groups=[list(range(N_CORES))],
                ins=[d2i.opt()], outs=[d2o.opt()])
            g2 = bcast_readback(d2o, 2, "g2")

            mu2, r2 = gn_mu_r(g2, 0, 1, N1, "2")
            al2 = stile([128, 1], "al2")
            nc.vector.tensor_mul(al2, r2, _col(pp, 4))
            be2 = stile([128, 1], "be2")
            nc.vector.tensor_mul(be2, mu2, al2)
            nc.vector.tensor_sub(be2, _col(pp, 5), be2)

            # ---- gelu(GN2) one-shot; accum_out is the SE partial sum ----
            m_col = stile([128, 1], "m_col")
            nc.scalar.activation(out=h2b, in_=h2,
                                 func=AF.Gelu, bias=be2, scale=al2,
                                 accum_out=m_col)
            # transpose [128,1] -> [1,128] via identity matmul: a
            # partition-strided SBUF->DRAM DMA does 128 scattered 4B reads
            # (~10us!) and stalls the AR3 trigger; a [1,128] row is one
            # contiguous burst.
            ps_t = ps.tile([1, 128], F32, tag="ps", name="ps_t")
            nc.tensor.matmul(ps_t, m_col, id_sb, start=True, stop=True)
            m_row = stile([1, 128], "m_row")
            nc.vector.tensor_copy(out=m_row, in_=ps_t)
            d3i = dram.tile([128], F32, name="d3i")
            d3o = dram.tile([128], F32, name="d3o")
            nc.sync.dma_start(out=d3i, in_=m_row)
            nc.gpsimd.collective_compute(
                "AllReduce", mybir.AluOpType.add,
                replica_groups=[list(range(N_CORES))],
                ins=[d3i.opt()], outs=[d3o.opt()])
            # read the 128-float result as one row, transpose back to a
            # column via PE (rhs = [1,1] one)
            m_row2 = stile([1, 128], "m_row2")
            nc.sync.dma_start(out=m_row2, in_=d3o)
            ps_mt = ps.tile([128, 1], F32, tag="ps", name="ps_mt")
            nc.tensor.matmul(ps_mt, m_row2, one_t, start=True, stop=True)
            m_sb = stile([128, 1], "m_sb")
            nc.vector.tensor_copy(out=m_sb, in_=ps_mt)

            # preload the Sigmoid table while the AR3 mesh runs (scalar
            # table cache holds ~2 entries; this evicts Copy, keeps Gelu)
            nc.scalar.activation(out=dummy, in_=dummy, func=AF.Sigmoid)

            # ---- SE MLP (tiny, replicated on every core) ----
            m_mean = stile([128, 1], "m_mean")
            nc.vector.tensor_scalar_mul(out=m_mean, in0=m_sb, scalar1=1.0 / P_SP)
            ps_se1 = ps.tile([8, 1], F32, tag="ps", name="ps_se1")
            nc.tensor.matmul(ps_se1, pp[:, 16:24], m_mean, start=True, stop=True)
            y1g = stile([8, 1], "y1g")
            nc.scalar.activation(out=y1g, in_=ps_se1, func=AF.Gelu)
            ps_se2 = ps.tile([128, 1], F32, tag="ps", name="ps_se2")
            nc.tensor.matmul(ps_se2, pp[0:8, 56:184], y1g, start=True, stop=True)
            s_sb = stile([128, 1], "s_sb")
            nc.scalar.activation(out=s_sb, in_=ps_se2, func=AF.Sigmoid)
            w3s = small.tile([128, 32], BF16, name="w3s")
            nc.vector.tensor_scalar_mul(out=w3s, in0=pp[:, 24:56], scalar1=s_sb)

            # ---- conv3 (bf16), 3-up packed: blocks n=3g+j land on
            # partition band 32j of PSUM group g -> 6 evictions instead of
            # 16. Band j>=1 of group 5 is zero-padded so the uniform
            # 3072-sample stats stay exact (zeros don't change sums).
            y3p = big.tile([96, 6 * 512], BF16, name="y3p", tag="h1slot")
            nc.vector.memset(y3p[32:64, 5 * 512:6 * 512], 0.0)
            nc.vector.memset(y3p[64:96, 5 * 512:6 * 512], 0.0)
            st3 = stile([96, 6, 6], "st3")
            for g in range(6):
                nj = 3 if g < 5 else 1
                pt3 = ps.tile([96, 512], F32, tag="ps", name=f"c3_{g}")
                for j in range(nj):
                    n = 3 * g + j
                    nc.tensor.matmul(pt3[32 * j:32 * j + 32, :], w3s,
                                     h2b[:, bass.ts(n, 512)],
                                     start=True, stop=True)
                blk = bass.ts(g, 512)
                nc.scalar.copy(out=y3p[0:32 * nj, blk], in_=pt3[0:32 * nj, :])
                nc.vector.bn_stats(out=st3[:, g, :], in_=y3p[0:96, blk])
            mv3 = stile([96, 2], "mv3")
            nc.vector.bn_aggr(out=mv3, in_=st3)
            pk3 = stile([128, 2], "pk3")
            nc.vector.memset(pk3, 0.0)
            NS3 = 6.0 * 512.0
            nc.vector.tensor_scalar_mul(out=pk3[0:96, 0:1], in0=mv3[:, 0:1], scalar1=NS3)
            t_g = sc("t_g")
            nc.vector.tensor_mul(t_g[0:96], mv3[:, 0:1], mv3[:, 0:1])
            nc.vector.tensor_add(t_g[0:96], t_g[0:96], mv3[:, 1:2])
            nc.vector.tensor_scalar_mul(out=pk3[0:96, 1:2], in0=t_g[0:96], scalar1=NS3)
            ps_s3 = ps.tile([1, 2], F32, tag="ps", name="ps_s3")
            nc.tensor.matmul(ps_s3, ones, pk3, start=True, stop=True)
            d4o = dram.tile([8], F32, name="d4o")
            row3 = stile([1, 2], "row3")
            nc.vector.tensor_copy(out=row3, in_=ps_s3)
            nc.sync.dma_start(out=d4i[0:2], in_=row3)
            nc.gpsimd.collective_compute(
                "AllReduce", mybir.AluOpType.add,
                replica_groups=[list(range(N_CORES))],
                ins=[d4i.opt()], outs=[d4o.opt()])
            g4 = bcast_readback(d4o, 2, "g4")

            mu3, r3 = gn_mu_r(g4, 0, 1, N3, "3")
            al3 = stile([128, 1], "al3")
            nc.vector.tensor_mul(al3, r3, _col(pp, 6))
            be3 = stile([128, 1], "be3")
            nc.vector.tensor_mul(be3, mu3, al3)
            nc.vector.tensor_sub(be3, _col(pp, 7), be3)

            # final affine on the packed layout (one DVE op), then three
            # band-unpack DMAs rebuild [32, POS] in DRAM. The f32 result
            # reuses aprime's SBUF slot. pp cols 6/7 hold gn3 w/b
            # replicated per 32-partition band.
            y3f = big.tile([96, 6 * 512], F32, name="y3f", tag="apslot")
            nc.vector.tensor_scalar(out=y3f, in0=y3p[0:96, :],
                                    scalar1=al3[0:96], scalar2=be3[0:96],
                                    op0=mybir.AluOpType.mult,
                                    op1=mybir.AluOpType.add)
            for j in range(3):
                ng = 6 if j == 0 else 5
                sb_ap = y3f[32 * j:32 * j + 32].rearrange(
                    "p (g i) -> p g i", g=6, i=512)[:, 0:ng, :]
                dram_ap = bass.AP(tensor=out_d.tensor,
                                  offset=out_d.offset + 512 * j,
                                  ap=[[POS, 32], [3 * 512, ng], [1, 512]])
                nc.sync.dma_start(out=dram_ap, in_=sb_ap)

    nc.compile()
    return nc


def _host_prep(inputs):
    x = np.asarray(inputs['x'], np.float32).reshape(CIN, S, S, S, S)
    g0w = np.asarray(inputs['g0_w'], np.float32)
    g0b = np.asarray(inputs['g0_b'], np.float32)
    W1 = np.asarray(inputs['w1'], np.float32).reshape(HID, CIN)
    gn1w = np.asarray(inputs['gn1_w'], np.float32)
    gn1b = np.asarray(inputs['gn1_b'], np.float32)
    w2 = np.asarray(inputs['w2'], np.float32).reshape(HID, HID, 3, 3, 3, 3)
    gn2w = np.asarray(inputs['gn2_w'], np.float32)
    gn2b = np.asarray(inputs['gn2_b'], np.float32)
    se1 = np.asarray(inputs['se_w1'], np.float32)   # [8,128]
    se2 = np.asarray(inputs['se_w2'], np.float32)   # [128,8]
    W3 = np.asarray(inputs['w3'], np.float32).reshape(CIN, HID)
    gn3w = np.asarray(inputs['gn3_w'], np.float32)
    gn3b = np.asarray(inputs['gn3_b'], np.float32)

    w1fold = W1 * g0w[None, :]
    w1rep = np.zeros((128, 128), np.float32)
    for j in range(4):
        w1rep[32 * j:32 * j + 32, :] = w1fold.T
    w1rep = w1rep.astype(ml_dtypes.bfloat16)
    u = W1 @ g0b
    v = W1 @ g0w
    w2t = np.ascontiguousarray(
        w2.transpose(1, 2, 3, 4, 5, 0).reshape(HID, 81 * HID)).astype(
            ml_dtypes.bfloat16)

    params = np.zeros((128, 192), np.float32)
    params[:, 0] = u
    params[:, 1] = v
    params[:, 2] = gn1w
    params[:, 3] = gn1b
    params[:, 4] = gn2w
    params[:, 5] = gn2b
    params[0:96, 6] = np.tile(gn3w, 3)
    params[0:96, 7] = np.tile(gn3b, 3)
    params[:, 10] = u.sum()
    params[:, 11] = v.sum()
    params[:, 12] = (u * u).sum()
    params[:, 13] = (u * v).sum()
    params[:, 14] = (v * v).sum()
    params[:, 16:24] = se1.T
    params[:, 24:56] = W3.T
    params[0:8, 56:184] = se2.T

    xp = np.zeros((CIN, S + 2, S, S, S), np.float32)
    xp[:, 1:S + 1] = x

    in_maps = []
    for k in range(N_CORES):
        p = params.copy()
        p[:, 8] = 0.0 if k == 0 else 1.0
        p[:, 9] = 0.0 if k == N_CORES - 1 else 1.0
        # stored plane order: [owned0, owned1, haloL, haloR]
        idx = [2 * k + 1, 2 * k + 2, 2 * k, 2 * k + 3]
        shard = np.ascontiguousarray(
            xp[:, idx].transpose(1, 0, 2, 3, 4).reshape(128, PLANE)).astype(
                ml_dtypes.bfloat16)
        in_maps.append({"xs": shard, "w1rep": w1rep, "w2t": w2t, "params": p,
                        "ident": np.eye(128, dtype=np.float32)})
    return in_maps


def kernel(**inputs):
    if "nc" not in _cache:
        _cache["nc"] = build_program()
    nc = _cache["nc"]
    in_maps = _host_prep(inputs)
    res = run_bass_kernel_spmd(nc, in_maps, core_ids=list(range(N_CORES)))
    out = np.empty((1, CIN, S, S, S, S), np.float32)
    for k in range(N_CORES):
        out[0, :, 2 * k:2 * k + 2] = res.results[k]["out"].reshape(CIN, 2, S, S, S)
    return out


def run_traced(inputs):
    """Like kernel() but with NTFF tracing; returns (out, BassKernelResults)."""
    if "nc" not in _cache:
        _cache["nc"] = build_program()
    nc = _cache["nc"]
    in_maps = _host_prep(inputs)
    res = run_bass_kernel_spmd(nc, in_maps, core_ids=list(range(N_CORES)),
                               trace=True)
    out = np.empty((1, CIN, S, S, S, S), np.float32)
    for k in range(N_CORES):
        out[0, :, 2 * k:2 * k + 2] = res.results[k]["out"].reshape(CIN, 2, S, S, S)
    return out, res
